# revision 2
# baseline (speedup 1.0000x reference)
"""Distributed Trainium2 kernel for nn_AncProbsLayer.

Math (reference):
    tau[b,h]  = softplus(tau_kernel[h, rate_indices[b,h]])
    R,p,Q     from tiny (H,K,20,20) kernels; Sm = D^1/2 Q D^-1/2; lam,U = eigh(Sm)
    P[b,h,k]  = D^-1/2 U diag(exp(tau*lam)) U^T D^1/2
    out       = einsum('blhz,bhkzs->blhks', inputs, P)

Device algorithm (V,W tiny host-precomputed eigen matrices; E from a
device-side indirect-DMA gather of tau_kernel + softplus + exp):
    P_comb[b]  = BDV @ (diag(E[b]) @ BDW)          (40x80, per-batch stationary)
    out[b,l,:] = in[b,l,:] @ P_comb[b]             (one matmul per batch)

Key precision facts exploited:
  * the inputs are EXACT one-hot vectors (0.0/1.0) -> fp8_e4m3 stream is
    exact, and no hi/lo input split is needed;
  * tolerance is 2e-2 absmax-relative -> a bf16 stationary (~2e-3 rel)
    and bf16 outputs (~2e-3) are comfortably accurate.
So the big matmul is ONE bf16-stationary x fp8-stream matmul per batch
(mixed non-fp32 operand dtypes are legal on the PE), fp32 PSUM.

Distribution: data-parallel over batch B across 8 cores (128 b each);
tiny kernels + tau table replicated (no collectives needed). Streams are
fed feature-major ([40, rows]) so the PE contracts over the partition dim.
Even/odd batches live on SBUF partitions 0-39 / 64-103 (disjoint PE row
groups via tile_position, both SBUF port halves covered by input DMA);
P_comb stationaries are built in 4 parity quarters directly at their
partition homes (column tile_position on the setup matmuls), PSUM is
evacuated in 2-batch tiles with the copy split DVE/ACT, input DMAs ride
the SP HWDGE ring while output DMAs ride the ACT ring to avoid
head-of-line blocking.
"""

import numpy as np
import ml_dtypes

import concourse.bass as bass
import concourse.bacc as bacc
import concourse.mybir as mybir
from concourse.tile import TileContext
from concourse.masks import make_identity
from concourse.bass_utils import run_bass_kernel_spmd

# Problem constants (hardcoded per the harness contract)
B, L, H, K, S = 1024, 512, 2, 2, 20
NUM_RATES = 100000
NCORES = 8
BPC = B // NCORES          # 128 batches per core
ROWS = BPC * L             # 65536 stream rows per core
HZ = H * S                 # 40  (input feature dim)
HKS = H * K * S            # 80  (output feature dim)
CB = 16                    # batches per DMA chunk
F32 = mybir.dt.float32
BF16 = mybir.dt.bfloat16
F8 = mybir.dt.float8e4
NPBF16 = np.dtype(ml_dtypes.bfloat16)
NPF8 = np.dtype(mybir.dt.np(F8))

_NC_CACHE = {}


def build_nc(reps=1):
    # reps>1 repeats the main stream inside one NEFF (benchmarking only:
    # (wall[R] - wall[1])/(R-1) cancels dispatch overhead exactly)
    nc = bacc.Bacc(
        "TRN2", target_bir_lowering=False, debug=False, num_devices=NCORES
    )
    # input pre-packed on host as a 128-partition image: rows 0-39 = even-b
    # features, rows 64-103 = odd-b, rest zero. 37% padding bytes, but
    # 128-partition DMAs run much faster than narrow ones (port binding),
    # and one DMA per chunk replaces four. fp8 one-hot is EXACT.
    in_img = nc.declare_dram_parameter("in_img", [128, ROWS // 2], F8, isOutput=False)
    tau_tab = nc.declare_dram_parameter("tau_tab", [H * NUM_RATES, 1], F32, isOutput=False)
    offs = nc.declare_dram_parameter("offs", [BPC, H], mybir.dt.int32, isOutput=False)
    bdvT = nc.declare_dram_parameter("bdvT", [HKS, HZ], F32, isOutput=False)
    bdw = nc.declare_dram_parameter("bdw", [HKS, HKS], F32, isOutput=False)
    lam_rep = nc.declare_dram_parameter("lam_rep", [BPC, HKS], F32, isOutput=False)
    out = nc.declare_dram_parameter("out", [HKS, ROWS], BF16, isOutput=True)

    QB = 32                    # batches per pc quarter
    NQ = BPC // QB             # 4 quarters
    with TileContext(nc) as tc:
        with (
            tc.tile_pool(name="const", bufs=1) as cpool,
            tc.tile_pool(name="setup", bufs=2) as spool,
            tc.tile_pool(name="inp", bufs=4) as ipool,
            tc.tile_pool(name="ost", bufs=3) as opool,
            tc.tile_pool(name="psE", bufs=1, space="PSUM") as psE,
            tc.tile_pool(name="psP", bufs=1, space="PSUM") as psP,
            tc.tile_pool(name="psO", bufs=3, space="PSUM") as psO,
        ):
            # ---- constants / setup ----
            bdvT_t = cpool.tile([HKS, HZ], dtype=F32)
            nc.sync.dma_start(out=bdvT_t[:], in_=bdvT[:])
            bdw_t = cpool.tile([HKS, HKS], dtype=F32)
            nc.sync.dma_start(out=bdw_t[:], in_=bdw[:])
            lam_t = cpool.tile([BPC, HKS], dtype=F32)
            nc.sync.dma_start(out=lam_t[:], in_=lam_rep[:])
            offs_t = cpool.tile([BPC, H], dtype=mybir.dt.int32)
            nc.sync.dma_start(out=offs_t[:], in_=offs[:])
            ident = cpool.tile([BPC, BPC], dtype=F32)
            make_identity(nc, ident[:])

            # ---- gather tau values: tau_raw[b,h] = tau_tab[offs[b,h]] ----
            tau_raw = cpool.tile([BPC, H], dtype=F32)
            for h in range(H):
                nc.gpsimd.indirect_dma_start(
                    out=tau_raw[:, h : h + 1],
                    out_offset=None,
                    in_=tau_tab[:],
                    in_offset=bass.IndirectOffsetOnAxis(
                        ap=offs_t[:, h : h + 1], axis=0
                    ),
                )
            # softplus(x) = ln(exp(x) + 1): the ACT table set
            # (natural_log_exp_and_others) has exp/ln/copy but no softplus.
            tau_ex = cpool.tile([BPC, H], dtype=F32)
            nc.scalar.activation(
                tau_ex[:], tau_raw[:], mybir.ActivationFunctionType.Exp
            )
            tau_sp = cpool.tile([BPC, H], dtype=F32)
            nc.scalar.activation(
                tau_sp[:], tau_ex[:], mybir.ActivationFunctionType.Ln, bias=1.0
            )

            # ---- E[b, hks] = exp(tau[b,h] * lam[hks]) ----
            E = cpool.tile([BPC, HKS], dtype=F32)
            for h in range(H):
                sl = slice(h * K * S, (h + 1) * K * S)
                nc.scalar.activation(
                    E[:, sl],
                    lam_t[:, sl],
                    mybir.ActivationFunctionType.Exp,
                    scale=tau_sp[:, h : h + 1],
                )
            # transpose E -> E_T [80, 128] so per-b columns are per-partition scalars
            e_ps = psE.tile([HKS, BPC], dtype=F32, space="PSUM")
            nc.tensor.transpose(out=e_ps[:], in_=E[:], identity=ident[:])
            e_t = cpool.tile([HKS, BPC], dtype=F32)
            nc.vector.tensor_copy(out=e_t[:], in_=e_ps[:])

            # ---- setup phase: P_comb bf16 stationaries, in 4 PARITY
            # quarters (q = half*2 + b%2). Odd-parity quarters are produced
            # directly at partitions 64-103 via column tile_position on the
            # small matmuls, so no cross-partition replication is needed.
            # bdwe[:, i*80+j] = BDW[:, j] * E_T[:, b(i)] via stride-0
            # broadcast APs; P_comb = BDV @ bdwe in batched fp32 matmuls.
            e_t4 = e_t[:].rearrange("p (hh i two) -> p hh two i", two=2, i=QB)
            pc_q = []
            for q in range(NQ):
                hh, par = q // 2, q % 2
                bdwe = spool.tile([HKS, QB * HKS], dtype=F32, tag="bdwe")
                nc.gpsimd.tensor_mul(
                    bdwe[:].rearrange("p (b j) -> p b j", j=HKS),
                    bdw_t[:, None, :].to_broadcast([HKS, QB, HKS]),
                    e_t4[:, hh, par, :].to_broadcast([HKS, QB, HKS]),
                )
                pc_t = cpool.tile([128, QB * HKS], dtype=BF16, tag=f"pc{q}")
                pb = 64 * par           # partition base for this parity
                tp = (0, 64) if par else None
                for m in range((QB * HKS) // L):
                    cs = slice(m * L, (m + 1) * L)
                    pc_ps = psP.tile([128, L], dtype=F32, space="PSUM", tag="pc")
                    nc.tensor.matmul(
                        pc_ps[pb : pb + HZ, cs.start - cs.start : L],
                        lhsT=bdvT_t[:], rhs=bdwe[:, cs],
                        start=True, stop=True, tile_position=tp,
                    )
                    nc.scalar.copy(
                        out=pc_t[pb : pb + HZ, cs], in_=pc_ps[pb : pb + HZ, :]
                    )
                pc_q.append(pc_t)

            # ---- main stream: 16 chunks x 16 batches (8 even/odd pairs) ----
            for _rep in range(reps):
              for ci in range(BPC // CB):
                  csl = slice(ci * (CB // 2) * L, (ci + 1) * (CB // 2) * L)
                  it = ipool.tile([128, (CB // 2) * L], dtype=F8, tag="it")
                  nc.sync.dma_start(out=it[:], in_=in_img[:, csl])
                  for jj in range(CB // 2):
                      # even/odd batch pair: even on PE rows 0-39, odd on rows
                      # 64-103 (disjoint row groups run concurrently); the two
                      # accumulation groups land in the 2 banks of one PSUM tile
                      be = ci * CB + jj * 2
                      bo = be + 1
                      qe = (be // 64) * 2 + (be % 2)
                      qo = (bo // 64) * 2 + (bo % 2)
                      bqe = (be % 64) // 2
                      bqo = (bo % 64) // 2
                      pse = slice(bqe * HKS, (bqe + 1) * HKS)
                      pso = slice(bqo * HKS, (bqo + 1) * HKS)
                      xs = slice(jj * L, (jj + 1) * L)
                      o_ps = psO.tile([HKS, 2 * L], dtype=F32, space="PSUM", tag="o")
                      oe = slice(0, L)
                      oo = slice(L, 2 * L)
                      HI = slice(64, 64 + HZ)
                      LO = slice(0, HZ)
                      nc.tensor.matmul(
                          o_ps[:, oe], lhsT=pc_q[qe][LO, pse], rhs=it[LO, xs],
                          start=True, stop=True,
                      )
                      nc.tensor.matmul(
                          o_ps[:, oo], lhsT=pc_q[qo][HI, pso], rhs=it[HI, xs],
                          start=True, stop=True,
                      )
                      # split the copy by columns: DVE is faster than ACT at
                      # f32 PSUM reads in practice, so DVE takes the bigger cut
                      CSPL = 340
                      if jj % 4 == 0:
                          ot4 = opool.tile([HKS, 8 * L], dtype=BF16)
                      ot = ot4[:, (jj % 4) * 2 * L : (jj % 4 + 1) * 2 * L]
                      o3 = o_ps[:].rearrange("p (g c) -> p g c", c=L)
                      ot3 = ot.rearrange("p (g c) -> p g c", c=L)
                      nc.vector.tensor_copy(
                          out=ot3[:, :, :CSPL], in_=o3[:, :, :CSPL]
                      )
                      nc.scalar.copy(out=ot3[:, :, CSPL:], in_=o3[:, :, CSPL:])
                      # out-DMA on the ACT HWDGE ring: keeps the SP ring free
                      # for input prefetch (no head-of-line blocking)
                      if jj % 4 == 3:
                          c0 = (ci * CB + (jj - 3) * 2) * L
                          nc.scalar.dma_start(
                              out=out[:, c0 : c0 + 8 * L], in_=ot4[:]
                          )
    nc.finalize()
    return nc


def _host_prep(exchangeability_kernel, equilibrium_kernel):
    """Tiny (H,K,20,20) eigen prep in float64 on host -> BDV, BDW, lam."""
    ek = exchangeability_kernel.astype(np.float64)
    eq = equilibrium_kernel.astype(np.float64)
    Rm = 0.5 * (ek + np.swapaxes(ek, -1, -2))
    Rm = np.logaddexp(0.0, Rm)  # softplus
    Rm = Rm * (1.0 - np.eye(S))
    # softmax
    em = eq - eq.max(axis=-1, keepdims=True)
    p = np.exp(em)
    p /= p.sum(axis=-1, keepdims=True)
    Q = Rm * p[..., None, :]
    row = Q.sum(axis=-1)
    Q = Q - row[..., :, None] * np.eye(S)
    mue = (p * row).sum(axis=-1)[..., None, None]
    Q = Q / np.maximum(mue, 1e-16)
    sqrt_p = np.sqrt(p)
    inv_sqrt_p = 1.0 / sqrt_p
    Sm = sqrt_p[..., :, None] * Q * inv_sqrt_p[..., None, :]
    Sm = 0.5 * (Sm + np.swapaxes(Sm, -1, -2))
    lam, U = np.linalg.eigh(Sm)  # (H,K,S), (H,K,S,S)

    BDV = np.zeros((HZ, HKS), dtype=np.float64)
    BDW = np.zeros((HKS, HKS), dtype=np.float64)
    for h in range(H):
        for k in range(K):
            c = h * K * S + k * S
            # V[z,s] = U[z,s]/sqrt(p[z]) ; rows = (h,z), cols = (h,k,s)
            BDV[h * S : (h + 1) * S, c : c + S] = inv_sqrt_p[h, k][:, None] * U[h, k]
            # BDW[(h,k,s),(h,k,j)] = sqrt(p[j]) * U[j,s]
            BDW[c : c + S, c : c + S] = (sqrt_p[h, k][:, None] * U[h, k]).T
    lam_flat = lam.reshape(HKS)
    return BDV.astype(np.float32), BDW.astype(np.float32), lam_flat.astype(np.float32)


def kernel(inputs, rate_indices, tau_kernel, exchangeability_kernel, equilibrium_kernel):
    inputs = np.asarray(inputs, dtype=np.float32)
    rate_indices = np.asarray(rate_indices)
    tau_kernel = np.asarray(tau_kernel, dtype=np.float32)

    BDV, BDW, lam_flat = _host_prep(
        np.asarray(exchangeability_kernel), np.asarray(equilibrium_kernel)
    )
    BDV_T = np.ascontiguousarray(BDV.T)
    lam_rep = np.broadcast_to(lam_flat, (BPC, HKS)).copy()
    tau_tab = tau_kernel.reshape(H * NUM_RATES, 1)

    if "nc" not in _NC_CACHE:
        _NC_CACHE["nc"] = build_nc()
    nc = _NC_CACHE["nc"]

    in_maps = []
    for c in range(NCORES):
        bsl = slice(c * BPC, (c + 1) * BPC)
        # feature-major stream layout: [40, 65536]; the one-hot values are
        # exactly representable in fp8_e4m3 (0.0 / 1.0), no precision loss
        inT_c = np.ascontiguousarray(inputs[bsl].reshape(BPC * L, HZ).T)
        f8 = inT_c.astype(NPF8)
        f83 = f8.reshape(HZ, BPC, L)
        img = np.zeros((128, ROWS // 2), dtype=NPF8)
        img[:HZ] = f83[:, 0::2].reshape(HZ, ROWS // 2)
        img[64 : 64 + HZ] = f83[:, 1::2].reshape(HZ, ROWS // 2)
        offs_c = (
            np.arange(H, dtype=np.int64)[None, :] * NUM_RATES
            + rate_indices[bsl].astype(np.int64)
        ).astype(np.int32)
        in_maps.append(
            {
                "in_img": img,
                "tau_tab": tau_tab,
                "offs": np.ascontiguousarray(offs_c),
                "bdvT": BDV_T,
                "bdw": BDW,
                "lam_rep": lam_rep,
            }
        )

    _NC_CACHE["in_maps"] = in_maps
    res = run_bass_kernel_spmd(nc, in_maps, core_ids=list(range(NCORES)))

    out = np.empty((B, L, H, K, S), dtype=np.float32)
    for c in range(NCORES):
        o = res.results[c]["out"]  # (80, 65536) bf16
        out[c * BPC : (c + 1) * BPC] = (
            o.astype(np.float32).T.reshape(BPC, L, H, K, S)
        )
    return out


# revision 8
# speedup vs baseline: 3.3114x; 3.3114x over previous
"""Distributed Trainium2 kernel for nn_AncProbsLayer.

Math (reference):
    tau[b,h]  = softplus(tau_kernel[h, rate_indices[b,h]])
    R,p,Q     from tiny (H,K,20,20) kernels; Sm = D^1/2 Q D^-1/2; lam,U = eigh(Sm)
    P[b,h,k]  = D^-1/2 U diag(exp(tau*lam)) U^T D^1/2
    out       = einsum('blhz,bhkzs->blhks', inputs, P)

Device algorithm (V,W tiny host-precomputed eigen matrices; E from a
device-side indirect-DMA gather of tau_kernel + softplus + exp):
    P_comb[b]  = BDV @ (diag(E[b]) @ BDW)          (40x80, per-batch)
    out[b,l,:] = in[b,l,:] @ P_comb[b]

Key facts exploited:
  * inputs are EXACT one-hot vectors (0.0/1.0) -> the fp8_e4m3 stream is
    exact and no hi/lo input split is needed. Measured on HW: an fp8
    moving operand streams at full pipelined rate (~81ns/matmul) while a
    bf16 moving operand runs ~7x slower - so the stream stays fp8.
  * tolerance is 2e-2 absmax-relative -> bf16 stationary (~2e-3 rel)
    and bf16 outputs (~2e-3) are comfortably accurate.

Dense-PSUM packing: each batch-pair (even b on image rows 0-39, odd b on
rows 64-103) is computed by TWO matmuls:
  MM1: a 128x128 stationary = [even-P at rows 0-39 -> cols 0-79;
       odd-P cols 0-47 at rows 64-103 -> cols 80-127]. Since the even
       and odd features occupy disjoint contraction rows and the unused
       stationary blocks are zero, one matmul yields a FULLY DENSE
       [128, 512] PSUM tile (no idle partitions).
  MM2: the odd batch's leftover outputs 48-79 (40x32 stationary at rows
       64-103), col-tiled to partition strip 32*(pair%4) of a shared
       [128, 512] "leftover" tile packing 4 pairs per tile.
This makes every PSUM-evict copy and every output DMA span all 128
partitions with zero padding: eviction cycle count (which is
free-dim-elements per partition on DVE/ACT) drops 1.6x, and the output
DMA engages all 16 SDMA ports. Eviction is column-split DVE/ACT in the
ratio of their clock rates.

Distribution: data-parallel over batch B across 8 cores (128 b each);
tiny kernels + tau table replicated (no collectives needed).
"""

import numpy as np
import ml_dtypes

import concourse.bass as bass
import concourse.bacc as bacc
import concourse.mybir as mybir
from concourse.tile import TileContext
from concourse.masks import make_identity
from concourse.bass_utils import run_bass_kernel_spmd

# Problem constants (hardcoded per the harness contract)
B, L, H, K, S = 1024, 512, 2, 2, 20
NUM_RATES = 100000
NCORES = 8
BPC = B // NCORES          # 128 batches per core
ROWS = BPC * L             # 65536 stream rows per core
HZ = H * S                 # 40  (input feature dim)
HKS = H * K * S            # 80  (output feature dim)
NPAIR = BPC // 2           # 64 batch pairs
CB = 16                    # batches per chunk (8 pairs)
NCHUNK = BPC // CB         # 8 chunks
PCW = 160                  # stationary cols per pair (128 MM1 + 32 MM2)
OCW = 10 * L               # output cols per chunk: 8 pair-slots + 2 leftover
F32 = mybir.dt.float32
BF16 = mybir.dt.bfloat16
F8 = mybir.dt.float8e4
NPBF16 = np.dtype(ml_dtypes.bfloat16)
NPF8 = np.dtype(mybir.dt.np(F8))

_NC_CACHE = {}


def build_nc(reps=1, timing=False):
    # reps>1 repeats the main stream inside one NEFF (benchmarking only:
    # (wall[R] - wall[1])/(R-1) cancels dispatch overhead exactly).
    # timing=True keeps the big output DRAM-internal (identical DMA work,
    # no host readback) so wall-clock deltas aren't noise-dominated.
    nc = bacc.Bacc(
        "TRN2", target_bir_lowering=False, debug=False, num_devices=NCORES
    )
    # input pre-packed on host as a 128-partition image: rows 0-39 = even-b
    # features, rows 64-103 = odd-b, rest zero (the zero rows meet zero
    # stationary blocks, so MM1 can contract over all 128 partitions).
    in_img = nc.declare_dram_parameter("in_img", [128, ROWS // 2], F8, isOutput=False)
    tau_tab = nc.declare_dram_parameter("tau_tab", [H * NUM_RATES, 1], F32, isOutput=False)
    offs = nc.declare_dram_parameter("offs", [BPC, H], mybir.dt.int32, isOutput=False)
    bdvT = nc.declare_dram_parameter("bdvT", [HKS, HZ], F32, isOutput=False)
    bdw = nc.declare_dram_parameter("bdw", [HKS, HKS], F32, isOutput=False)
    lam_rep = nc.declare_dram_parameter("lam_rep", [BPC, HKS], F32, isOutput=False)
    if timing:
        out = nc.dram_tensor("out", [128, NCHUNK * OCW], BF16, kind="Internal")
        outd = nc.declare_dram_parameter("outd", [128, 128], F32, isOutput=True)
    else:
        out = nc.declare_dram_parameter("out", [128, NCHUNK * OCW], BF16, isOutput=True)

    QB = 32                    # batches per pc quarter
    NQ = BPC // QB             # 4 quarters
    SC = 320                   # setup matmul free size (4 batches x 80)
    with TileContext(nc) as tc:
        with (
            tc.tile_pool(name="const", bufs=1) as cpool,
            tc.tile_pool(name="setup", bufs=2) as spool,
            tc.tile_pool(name="inp", bufs=4) as ipool,
            tc.tile_pool(name="ost", bufs=3) as opool,
        ):
            # ---- constants / setup ----
            bdvT_t = cpool.tile([HKS, HZ], dtype=F32)
            nc.sync.dma_start(out=bdvT_t[:], in_=bdvT[:])
            bdw_t = cpool.tile([HKS, HKS], dtype=F32)
            nc.sync.dma_start(out=bdw_t[:], in_=bdw[:])
            lam_t = cpool.tile([BPC, HKS], dtype=F32)
            nc.sync.dma_start(out=lam_t[:], in_=lam_rep[:])
            offs_t = cpool.tile([BPC, H], dtype=mybir.dt.int32)
            nc.sync.dma_start(out=offs_t[:], in_=offs[:])
            ident = cpool.tile([BPC, BPC], dtype=F32)
            make_identity(nc, ident[:])

            # combined per-pair stationaries, zeroed once (the zero blocks
            # are load-bearing: they mask the cross-parity terms of MM1)
            pcAll = cpool.tile([128, NPAIR * PCW], dtype=BF16)
            nc.gpsimd.memset(pcAll[:], 0.0)
            pcv = pcAll[:].rearrange("p (pr c) -> p pr c", c=PCW)

            # ---- gather tau values: tau_raw[b,h] = tau_tab[offs[b,h]] ----
            tau_raw = cpool.tile([BPC, H], dtype=F32)
            for h in range(H):
                nc.gpsimd.indirect_dma_start(
                    out=tau_raw[:, h : h + 1],
                    out_offset=None,
                    in_=tau_tab[:],
                    in_offset=bass.IndirectOffsetOnAxis(
                        ap=offs_t[:, h : h + 1], axis=0
                    ),
                )
            # softplus(x) = ln(exp(x) + 1): the ACT table set
            # (natural_log_exp_and_others) has exp/ln/copy but no softplus.
            tau_ex = cpool.tile([BPC, H], dtype=F32)
            nc.scalar.activation(
                tau_ex[:], tau_raw[:], mybir.ActivationFunctionType.Exp
            )
            tau_sp = cpool.tile([BPC, H], dtype=F32)
            nc.scalar.activation(
                tau_sp[:], tau_ex[:], mybir.ActivationFunctionType.Ln, bias=1.0
            )

            # ---- E[b, hks] = exp(tau[b,h] * lam[hks]) ----
            E = cpool.tile([BPC, HKS], dtype=F32)
            for h in range(H):
                sl = slice(h * K * S, (h + 1) * K * S)
                nc.scalar.activation(
                    E[:, sl],
                    lam_t[:, sl],
                    mybir.ActivationFunctionType.Exp,
                    scale=tau_sp[:, h : h + 1],
                )

            with (
                tc.tile_pool(name="psE", bufs=1, space="PSUM") as psE,
                tc.tile_pool(name="psP", bufs=2, space="PSUM") as psP,
            ):
                # transpose E -> E_T [80, 128]: per-b columns become
                # per-partition scalars
                e_ps = psE.tile([HKS, BPC], dtype=F32, space="PSUM")
                nc.tensor.transpose(out=e_ps[:], in_=E[:], identity=ident[:])
                e_t = cpool.tile([HKS, BPC], dtype=F32)
                nc.vector.tensor_copy(out=e_t[:], in_=e_ps[:])

                # ---- P_comb stationaries in 4 PARITY quarters (q =
                # half*2 + b%2); odd-parity P lands at partitions 64-103 via
                # column tile_position on the small matmuls.
                # bdwe[:, i*80+j] = BDW[:, j] * E_T[:, b(i)];
                # P_comb = BDV @ bdwe in batched fp32 matmuls of 4 batches,
                # then scattered into the per-pair stationary layout:
                #   even-P -> pcv[0:40, pair, 0:80]
                #   odd-P  -> pcv[64:104, pair, 80:160]
                e_t4 = e_t[:].rearrange("p (hh i two) -> p hh two i", two=2, i=QB)
                for q in range(NQ):
                    hh, par = q // 2, q % 2
                    bdwe = spool.tile([HKS, QB * HKS], dtype=F32, tag="bdwe")
                    nc.gpsimd.tensor_mul(
                        bdwe[:].rearrange("p (b j) -> p b j", j=HKS),
                        bdw_t[:, None, :].to_broadcast([HKS, QB, HKS]),
                        e_t4[:, hh, par, :].to_broadcast([HKS, QB, HKS]),
                    )
                    pb = 64 * par           # partition base for this parity
                    tp = (0, 64) if par else None
                    for m in range((QB * HKS) // SC):
                        cs = slice(m * SC, (m + 1) * SC)
                        pc_ps = psP.tile([128, SC], dtype=F32, space="PSUM", tag="pc")
                        nc.tensor.matmul(
                            pc_ps[pb : pb + HZ, 0:SC],
                            lhsT=bdvT_t[:], rhs=bdwe[:, cs],
                            start=True, stop=True, tile_position=tp,
                        )
                        prs = slice(hh * QB + m * 4, hh * QB + m * 4 + 4)
                        src = pc_ps[pb : pb + HZ, 0:SC].rearrange(
                            "p (b c) -> p b c", c=HKS
                        )
                        if par == 0:
                            nc.scalar.copy(out=pcv[0:HZ, prs, 0:HKS], in_=src)
                        else:
                            nc.scalar.copy(
                                out=pcv[64 : 64 + HZ, prs, HKS:PCW], in_=src
                            )

            # ---- main stream: 8 chunks x 8 pairs ----
            # All PSUM eviction runs on DVE alone: HW probing showed a
            # DVE-only eviction stream pipelines cleanly with the matmuls
            # (bf16-dest copies ~2x) while mixing ACT copies into the loop
            # serializes it ~10x. ACT's only main-loop job is issuing the
            # output DMAs (HWDGE ring), SP's only job the input DMAs.
            with (
                tc.tile_pool(name="psA", bufs=3, space="PSUM") as psA,
                tc.tile_pool(name="psL", bufs=2, space="PSUM") as psL,
            ):
              for _rep in range(reps):
                for ci in range(NCHUNK):
                    csl = slice(ci * (CB // 2) * L, (ci + 1) * (CB // 2) * L)
                    it = ipool.tile([128, (CB // 2) * L], dtype=F8, tag="it")
                    nc.sync.dma_start(out=it[:], in_=in_img[:, csl])
                    ot = opool.tile([128, OCW], dtype=BF16)
                    for jj in range(CB // 2):
                        p = ci * (CB // 2) + jj      # global pair index
                        xs = slice(jj * L, (jj + 1) * L)
                        if jj % 2 == 0:
                            oa = psA.tile([128, 2 * L], dtype=F32, space="PSUM", tag="a")
                        if jj % 4 == 0:
                            ol = psL.tile([128, L], dtype=F32, space="PSUM", tag="l")
                        ob = slice((jj % 2) * L, (jj % 2) * L + L)
                        nc.tensor.matmul(
                            oa[:, ob],
                            lhsT=pcv[0:128, p, 0:128],
                            rhs=it[:, xs],
                            start=True, stop=True,
                        )
                        sq = 32 * (jj % 4)
                        nc.tensor.matmul(
                            ol[sq : sq + 32, 0:L],
                            lhsT=pcv[64 : 64 + HZ, p, 128:PCW],
                            rhs=it[64 : 64 + HZ, xs],
                            start=True, stop=True,
                            tile_position=(64, sq),
                        )
                        if jj % 2 == 1:
                            oc = slice((jj - 1) * L, (jj + 1) * L)
                            nc.vector.tensor_copy(out=ot[:, oc], in_=oa[:])
                        if jj % 4 == 3:
                            lc = slice(8 * L + (jj // 4) * L, 8 * L + (jj // 4 + 1) * L)
                            nc.vector.tensor_copy(out=ot[:, lc], in_=ol[:])
                    # single dense 128-partition output DMA per chunk
                    nc.scalar.dma_start(
                        out=out[:, ci * OCW : (ci + 1) * OCW], in_=ot[:]
                    )
              if timing:
                  nc.sync.dma_start(out=outd[:], in_=ident[:])
    nc.finalize()
    return nc


def _host_prep(exchangeability_kernel, equilibrium_kernel):
    """Tiny (H,K,20,20) eigen prep in float64 on host -> BDV, BDW, lam."""
    ek = exchangeability_kernel.astype(np.float64)
    eq = equilibrium_kernel.astype(np.float64)
    Rm = 0.5 * (ek + np.swapaxes(ek, -1, -2))
    Rm = np.logaddexp(0.0, Rm)  # softplus
    Rm = Rm * (1.0 - np.eye(S))
    # softmax
    em = eq - eq.max(axis=-1, keepdims=True)
    p = np.exp(em)
    p /= p.sum(axis=-1, keepdims=True)
    Q = Rm * p[..., None, :]
    row = Q.sum(axis=-1)
    Q = Q - row[..., :, None] * np.eye(S)
    mue = (p * row).sum(axis=-1)[..., None, None]
    Q = Q / np.maximum(mue, 1e-16)
    sqrt_p = np.sqrt(p)
    inv_sqrt_p = 1.0 / sqrt_p
    Sm = sqrt_p[..., :, None] * Q * inv_sqrt_p[..., None, :]
    Sm = 0.5 * (Sm + np.swapaxes(Sm, -1, -2))
    lam, U = np.linalg.eigh(Sm)  # (H,K,S), (H,K,S,S)

    BDV = np.zeros((HZ, HKS), dtype=np.float64)
    BDW = np.zeros((HKS, HKS), dtype=np.float64)
    for h in range(H):
        for k in range(K):
            c = h * K * S + k * S
            # V[z,s] = U[z,s]/sqrt(p[z]) ; rows = (h,z), cols = (h,k,s)
            BDV[h * S : (h + 1) * S, c : c + S] = inv_sqrt_p[h, k][:, None] * U[h, k]
            # BDW[(h,k,s),(h,k,j)] = sqrt(p[j]) * U[j,s]
            BDW[c : c + S, c : c + S] = (sqrt_p[h, k][:, None] * U[h, k]).T
    lam_flat = lam.reshape(HKS)
    return BDV.astype(np.float32), BDW.astype(np.float32), lam_flat.astype(np.float32)


def kernel(inputs, rate_indices, tau_kernel, exchangeability_kernel, equilibrium_kernel):
    inputs = np.asarray(inputs, dtype=np.float32)
    rate_indices = np.asarray(rate_indices)
    tau_kernel = np.asarray(tau_kernel, dtype=np.float32)

    BDV, BDW, lam_flat = _host_prep(
        np.asarray(exchangeability_kernel), np.asarray(equilibrium_kernel)
    )
    BDV_T = np.ascontiguousarray(BDV.T)
    lam_rep = np.broadcast_to(lam_flat, (BPC, HKS)).copy()
    tau_tab = tau_kernel.reshape(H * NUM_RATES, 1)

    if "nc" not in _NC_CACHE:
        _NC_CACHE["nc"] = build_nc()
    nc = _NC_CACHE["nc"]

    in_maps = []
    for c in range(NCORES):
        bsl = slice(c * BPC, (c + 1) * BPC)
        # feature-major stream layout: [40, 65536]; one-hot values are
        # exactly representable in fp8_e4m3 (0.0 / 1.0), no precision loss
        inT_c = np.ascontiguousarray(inputs[bsl].reshape(BPC * L, HZ).T)
        f8 = inT_c.astype(NPF8)
        f83 = f8.reshape(HZ, BPC, L)
        img = np.zeros((128, ROWS // 2), dtype=NPF8)
        img[:HZ] = f83[:, 0::2].reshape(HZ, ROWS // 2)
        img[64 : 64 + HZ] = f83[:, 1::2].reshape(HZ, ROWS // 2)
        offs_c = (
            np.arange(H, dtype=np.int64)[None, :] * NUM_RATES
            + rate_indices[bsl].astype(np.int64)
        ).astype(np.int32)
        in_maps.append(
            {
                "in_img": img,
                "tau_tab": tau_tab,
                "offs": np.ascontiguousarray(offs_c),
                "bdvT": BDV_T,
                "bdw": BDW,
                "lam_rep": lam_rep,
            }
        )

    _NC_CACHE["in_maps"] = in_maps
    res = run_bass_kernel_spmd(nc, in_maps, core_ids=list(range(NCORES)))

    out = np.empty((B, L, H, K, S), dtype=np.float32)
    for c in range(NCORES):
        o = res.results[c]["out"].astype(np.float32)   # [128, 40960]
        oc = o.reshape(128, NCHUNK, 10, L)             # part, chunk, slot, l
        # slots 0-7: dense pair slabs (slot == pair-in-chunk);
        # slots 8-9: leftover tiles for pairs 0-3 / 4-7
        A = oc[:, :, 0:8, :]
        even = A[0:HKS].transpose(1, 2, 3, 0)          # [chunk, pair, l, 80]
        oddlo = A[HKS:128].transpose(1, 2, 3, 0)       # [chunk, pair, l, 48]
        lam4 = oc[:, :, 8:10, :].reshape(4, 32, NCHUNK, 2, L)
        # [strip q, i, chunk, half, l]: pair = half*4 + q, odd feat 48+i
        oddhi = lam4.transpose(2, 3, 0, 4, 1)          # [chunk, half, q, l, 32]
        res_c = np.empty((BPC, L, HKS), dtype=np.float32)
        res_c[0::2] = even.reshape(BPC // 2, L, HKS)
        res_c[1::2, :, 0:48] = oddlo.reshape(BPC // 2, L, 48)
        res_c[1::2, :, 48:HKS] = oddhi.reshape(BPC // 2, L, 32)
        out[c * BPC : (c + 1) * BPC] = res_c.reshape(BPC, L, H, K, S)
    return out


# revision 10
# speedup vs baseline: 10.8943x; 3.2899x over previous
"""Distributed Trainium2 kernel for nn_AncProbsLayer.

Math (reference):
    tau[b,h]  = softplus(tau_kernel[h, rate_indices[b,h]])
    R,p,Q     from tiny (H,K,20,20) kernels; Sm = D^1/2 Q D^-1/2; lam,U = eigh(Sm)
    P[b,h,k]  = D^-1/2 U diag(exp(tau*lam)) U^T D^1/2
    out       = einsum('blhz,bhkzs->blhks', inputs, P)

Device algorithm (V,W tiny host-precomputed eigen matrices; E from a
device-side indirect-DMA gather of tau_kernel + softplus + exp):
    P_comb[b]  = BDV @ (diag(E[b]) @ BDW)          (40x80, per-batch stationary)
    out[b,l,:] = in[b,l,:] @ P_comb[b]             (one matmul per batch)

HW-probed facts this version is built on (vs the previous 3-matmul
bf16 hi/lo kernel at ~150.8us/pass):
  * the inputs are EXACT one-hot vectors (0.0/1.0): an fp8_e4m3 stream
    is exact, so the hi/lo input split is unnecessary. Probing showed
    fp8-stream matmuls pipeline ~7x faster than bf16-stream ones
    (~81ns vs ~594ns per 512-col matmul), so the whole stream side
    runs fp8: 1 matmul per batch instead of 3, and half the input DMA.
  * tolerance is 2e-2 absmax-relative: a bf16 stationary (~2e-3 rel
    error) and bf16 outputs (~2e-3) are comfortably accurate -> output
    DMA is halved too (bf16 instead of fp32).
  * PSUM eviction must be DVE-ONLY whole-tile copies: mixing ACT
    (scalar) copies into the loop - as the old kernel's column-split
    eviction did - serializes the pipeline ~10x (HW-probed). A pure
    DVE eviction stream ([80,1024] fp32->bf16 per pair) runs at
    ~0.58us/pair and overlaps the matmuls cleanly. ACT's only
    main-loop job is issuing output DMAs, SP's only job input DMAs.

Distribution: data-parallel over batch B across 8 cores (128 b each);
tiny kernels + tau table replicated (no collectives needed). Streams
are fed feature-major ([40, rows]); even/odd batches live on SBUF
partitions 0-39 / 64-103 (disjoint PE row groups run concurrently).
"""

import numpy as np
import ml_dtypes

import concourse.bass as bass
import concourse.bacc as bacc
import concourse.mybir as mybir
from concourse.tile import TileContext
from concourse.masks import make_identity
from concourse.bass_utils import run_bass_kernel_spmd

# Problem constants (hardcoded per the harness contract)
B, L, H, K, S = 1024, 512, 2, 2, 20
NUM_RATES = 100000
NCORES = 8
BPC = B // NCORES          # 128 batches per core
ROWS = BPC * L             # 65536 stream rows per core
HZ = H * S                 # 40  (input feature dim)
HKS = H * K * S            # 80  (output feature dim)
CB = 16                    # batches per DMA chunk
F32 = mybir.dt.float32
BF16 = mybir.dt.bfloat16
F8 = mybir.dt.float8e4
NPBF16 = np.dtype(ml_dtypes.bfloat16)
NPF8 = np.dtype(mybir.dt.np(F8))

_NC_CACHE = {}


def build_nc(reps=1, timing=False):
    # reps>1 repeats the main stream inside one NEFF (benchmarking only:
    # (wall[R] - wall[1])/(R-1) cancels dispatch overhead exactly).
    # timing=True keeps the big output DRAM-internal (identical DMA work,
    # no host readback) so wall-clock deltas aren't noise-dominated.
    nc = bacc.Bacc(
        "TRN2", target_bir_lowering=False, debug=False, num_devices=NCORES
    )
    # input pre-packed on host as a 128-partition image: rows 0-39 = even-b
    # features, rows 64-103 = odd-b, rest zero. fp8 one-hot is EXACT.
    # (timing builds keep the big tensors DRAM-internal: identical DMA
    # work on garbage data, but no per-run host transfer -> low noise)
    if timing:
        in_img = nc.dram_tensor("in_img", [128, ROWS // 2], F8, kind="Internal")
        tau_tab = nc.dram_tensor("tau_tab", [H * NUM_RATES, 1], F32, kind="Internal")
    else:
        in_img = nc.declare_dram_parameter("in_img", [128, ROWS // 2], F8, isOutput=False)
        tau_tab = nc.declare_dram_parameter("tau_tab", [H * NUM_RATES, 1], F32, isOutput=False)
    offs = nc.declare_dram_parameter("offs", [BPC, H], mybir.dt.int32, isOutput=False)
    bdvT = nc.declare_dram_parameter("bdvT", [HKS, HZ], F32, isOutput=False)
    bdw = nc.declare_dram_parameter("bdw", [HKS, HKS], F32, isOutput=False)
    lam_rep = nc.declare_dram_parameter("lam_rep", [BPC, HKS], F32, isOutput=False)
    if timing:
        out = nc.dram_tensor("out", [HKS, ROWS], BF16, kind="Internal")
        outd = nc.declare_dram_parameter("outd", [128, 128], F32, isOutput=True)
    else:
        out = nc.declare_dram_parameter("out", [HKS, ROWS], BF16, isOutput=True)

    QB = 32                    # batches per pc quarter
    NQ = BPC // QB             # 4 quarters
    with TileContext(nc) as tc:
        with (
            tc.tile_pool(name="const", bufs=1) as cpool,
            tc.tile_pool(name="setup", bufs=2) as spool,
            tc.tile_pool(name="inp", bufs=4) as ipool,
            tc.tile_pool(name="ost", bufs=3) as opool,
            tc.tile_pool(name="psE", bufs=1, space="PSUM") as psE,
            tc.tile_pool(name="psP", bufs=1, space="PSUM") as psP,
            tc.tile_pool(name="psO", bufs=3, space="PSUM") as psO,
        ):
            # ---- constants / setup ----
            bdvT_t = cpool.tile([HKS, HZ], dtype=F32)
            nc.sync.dma_start(out=bdvT_t[:], in_=bdvT[:])
            bdw_t = cpool.tile([HKS, HKS], dtype=F32)
            nc.sync.dma_start(out=bdw_t[:], in_=bdw[:])
            lam_t = cpool.tile([BPC, HKS], dtype=F32)
            nc.sync.dma_start(out=lam_t[:], in_=lam_rep[:])
            offs_t = cpool.tile([BPC, H], dtype=mybir.dt.int32)
            nc.sync.dma_start(out=offs_t[:], in_=offs[:])
            ident = cpool.tile([BPC, BPC], dtype=F32)
            make_identity(nc, ident[:])

            # ---- gather tau values: tau_raw[b,h] = tau_tab[offs[b,h]] ----
            tau_raw = cpool.tile([BPC, H], dtype=F32)
            for h in range(H):
                nc.gpsimd.indirect_dma_start(
                    out=tau_raw[:, h : h + 1],
                    out_offset=None,
                    in_=tau_tab[:],
                    in_offset=bass.IndirectOffsetOnAxis(
                        ap=offs_t[:, h : h + 1], axis=0
                    ),
                )
            # softplus(x) = ln(exp(x) + 1): the ACT table set
            # (natural_log_exp_and_others) has exp/ln/copy but no softplus.
            tau_ex = cpool.tile([BPC, H], dtype=F32)
            nc.scalar.activation(
                tau_ex[:], tau_raw[:], mybir.ActivationFunctionType.Exp
            )
            tau_sp = cpool.tile([BPC, H], dtype=F32)
            nc.scalar.activation(
                tau_sp[:], tau_ex[:], mybir.ActivationFunctionType.Ln, bias=1.0
            )

            # ---- E[b, hks] = exp(tau[b,h] * lam[hks]) ----
            E = cpool.tile([BPC, HKS], dtype=F32)
            for h in range(H):
                sl = slice(h * K * S, (h + 1) * K * S)
                nc.scalar.activation(
                    E[:, sl],
                    lam_t[:, sl],
                    mybir.ActivationFunctionType.Exp,
                    scale=tau_sp[:, h : h + 1],
                )
            # transpose E -> E_T [80, 128] so per-b columns are per-partition scalars
            e_ps = psE.tile([HKS, BPC], dtype=F32, space="PSUM")
            nc.tensor.transpose(out=e_ps[:], in_=E[:], identity=ident[:])
            e_t = cpool.tile([HKS, BPC], dtype=F32)
            nc.vector.tensor_copy(out=e_t[:], in_=e_ps[:])

            # ---- setup phase: P_comb bf16 stationaries, in 4 PARITY
            # quarters (q = half*2 + b%2). Odd-parity quarters are produced
            # directly at partitions 64-103 via column tile_position on the
            # small matmuls, so no cross-partition replication is needed.
            # bdwe[:, i*80+j] = BDW[:, j] * E_T[:, b(i)] via stride-0
            # broadcast APs; P_comb = BDV @ bdwe in batched fp32 matmuls.
            e_t4 = e_t[:].rearrange("p (hh i two) -> p hh two i", two=2, i=QB)
            pc_q = []
            for q in range(NQ):
                hh, par = q // 2, q % 2
                bdwe = spool.tile([HKS, QB * HKS], dtype=F32, tag="bdwe")
                nc.gpsimd.tensor_mul(
                    bdwe[:].rearrange("p (b j) -> p b j", j=HKS),
                    bdw_t[:, None, :].to_broadcast([HKS, QB, HKS]),
                    e_t4[:, hh, par, :].to_broadcast([HKS, QB, HKS]),
                )
                pc_t = cpool.tile([128, QB * HKS], dtype=BF16, tag=f"pc{q}")
                pb = 64 * par           # partition base for this parity
                tp = (0, 64) if par else None
                for m in range((QB * HKS) // L):
                    cs = slice(m * L, (m + 1) * L)
                    pc_ps = psP.tile([128, L], dtype=F32, space="PSUM", tag="pc")
                    nc.tensor.matmul(
                        pc_ps[pb : pb + HZ, 0:L],
                        lhsT=bdvT_t[:], rhs=bdwe[:, cs],
                        start=True, stop=True, tile_position=tp,
                    )
                    nc.scalar.copy(
                        out=pc_t[pb : pb + HZ, cs], in_=pc_ps[pb : pb + HZ, :]
                    )
                pc_q.append(pc_t)

            # ---- main stream: 8 chunks x 16 batches (8 even/odd pairs) ----
            for _rep in range(reps):
              for ci in range(BPC // CB):
                  csl = slice(ci * (CB // 2) * L, (ci + 1) * (CB // 2) * L)
                  it = ipool.tile([128, (CB // 2) * L], dtype=F8, tag="it")
                  nc.sync.dma_start(out=it[:], in_=in_img[:, csl])
                  for jj in range(CB // 2):
                      # even/odd batch pair: even on PE rows 0-39, odd on rows
                      # 64-103 (disjoint row groups run concurrently); the two
                      # accumulation groups land in the 2 banks of one PSUM tile
                      be = ci * CB + jj * 2
                      bo = be + 1
                      qe = (be // 64) * 2 + (be % 2)
                      qo = (bo // 64) * 2 + (bo % 2)
                      bqe = (be % 64) // 2
                      bqo = (bo % 64) // 2
                      pse = slice(bqe * HKS, (bqe + 1) * HKS)
                      pso = slice(bqo * HKS, (bqo + 1) * HKS)
                      xs = slice(jj * L, (jj + 1) * L)
                      o_ps = psO.tile([HKS, 2 * L], dtype=F32, space="PSUM", tag="o")
                      oe = slice(0, L)
                      oo = slice(L, 2 * L)
                      HI = slice(64, 64 + HZ)
                      LO = slice(0, HZ)
                      nc.tensor.matmul(
                          o_ps[:, oe], lhsT=pc_q[qe][LO, pse], rhs=it[LO, xs],
                          start=True, stop=True,
                      )
                      nc.tensor.matmul(
                          o_ps[:, oo], lhsT=pc_q[qo][HI, pso], rhs=it[HI, xs],
                          start=True, stop=True,
                      )
                      # whole-tile DVE-only eviction (fp32 PSUM -> bf16 SBUF)
                      if jj % 4 == 0:
                          ot4 = opool.tile([HKS, 8 * L], dtype=BF16)
                      nc.vector.tensor_copy(
                          out=ot4[:, (jj % 4) * 2 * L : (jj % 4 + 1) * 2 * L],
                          in_=o_ps[:],
                      )
                      # out-DMA on the ACT HWDGE ring: keeps the SP ring free
                      # for input prefetch (no head-of-line blocking)
                      if jj % 4 == 3:
                          c0 = (ci * CB + (jj - 3) * 2) * L
                          nc.scalar.dma_start(
                              out=out[:, c0 : c0 + 8 * L], in_=ot4[:]
                          )
              if timing:
                  nc.sync.dma_start(out=outd[:], in_=ident[:])
    nc.finalize()
    return nc


def _host_prep(exchangeability_kernel, equilibrium_kernel):
    """Tiny (H,K,20,20) eigen prep in float64 on host -> BDV, BDW, lam."""
    ek = exchangeability_kernel.astype(np.float64)
    eq = equilibrium_kernel.astype(np.float64)
    Rm = 0.5 * (ek + np.swapaxes(ek, -1, -2))
    Rm = np.logaddexp(0.0, Rm)  # softplus
    Rm = Rm * (1.0 - np.eye(S))
    # softmax
    em = eq - eq.max(axis=-1, keepdims=True)
    p = np.exp(em)
    p /= p.sum(axis=-1, keepdims=True)
    Q = Rm * p[..., None, :]
    row = Q.sum(axis=-1)
    Q = Q - row[..., :, None] * np.eye(S)
    mue = (p * row).sum(axis=-1)[..., None, None]
    Q = Q / np.maximum(mue, 1e-16)
    sqrt_p = np.sqrt(p)
    inv_sqrt_p = 1.0 / sqrt_p
    Sm = sqrt_p[..., :, None] * Q * inv_sqrt_p[..., None, :]
    Sm = 0.5 * (Sm + np.swapaxes(Sm, -1, -2))
    lam, U = np.linalg.eigh(Sm)  # (H,K,S), (H,K,S,S)

    BDV = np.zeros((HZ, HKS), dtype=np.float64)
    BDW = np.zeros((HKS, HKS), dtype=np.float64)
    for h in range(H):
        for k in range(K):
            c = h * K * S + k * S
            # V[z,s] = U[z,s]/sqrt(p[z]) ; rows = (h,z), cols = (h,k,s)
            BDV[h * S : (h + 1) * S, c : c + S] = inv_sqrt_p[h, k][:, None] * U[h, k]
            # BDW[(h,k,s),(h,k,j)] = sqrt(p[j]) * U[j,s]
            BDW[c : c + S, c : c + S] = (sqrt_p[h, k][:, None] * U[h, k]).T
    lam_flat = lam.reshape(HKS)
    return BDV.astype(np.float32), BDW.astype(np.float32), lam_flat.astype(np.float32)


def kernel(inputs, rate_indices, tau_kernel, exchangeability_kernel, equilibrium_kernel):
    inputs = np.asarray(inputs, dtype=np.float32)
    rate_indices = np.asarray(rate_indices)
    tau_kernel = np.asarray(tau_kernel, dtype=np.float32)

    BDV, BDW, lam_flat = _host_prep(
        np.asarray(exchangeability_kernel), np.asarray(equilibrium_kernel)
    )
    BDV_T = np.ascontiguousarray(BDV.T)
    lam_rep = np.broadcast_to(lam_flat, (BPC, HKS)).copy()
    tau_tab = tau_kernel.reshape(H * NUM_RATES, 1)

    if "nc" not in _NC_CACHE:
        _NC_CACHE["nc"] = build_nc()
    nc = _NC_CACHE["nc"]

    in_maps = []
    for c in range(NCORES):
        bsl = slice(c * BPC, (c + 1) * BPC)
        # feature-major stream layout: [40, 65536]; the one-hot values are
        # exactly representable in fp8_e4m3 (0.0 / 1.0), no precision loss
        inT_c = np.ascontiguousarray(inputs[bsl].reshape(BPC * L, HZ).T)
        f8 = inT_c.astype(NPF8)
        f83 = f8.reshape(HZ, BPC, L)
        img = np.zeros((128, ROWS // 2), dtype=NPF8)
        img[:HZ] = f83[:, 0::2].reshape(HZ, ROWS // 2)
        img[64 : 64 + HZ] = f83[:, 1::2].reshape(HZ, ROWS // 2)
        offs_c = (
            np.arange(H, dtype=np.int64)[None, :] * NUM_RATES
            + rate_indices[bsl].astype(np.int64)
        ).astype(np.int32)
        in_maps.append(
            {
                "in_img": img,
                "tau_tab": tau_tab,
                "offs": np.ascontiguousarray(offs_c),
                "bdvT": BDV_T,
                "bdw": BDW,
                "lam_rep": lam_rep,
            }
        )

    _NC_CACHE["in_maps"] = in_maps
    res = run_bass_kernel_spmd(nc, in_maps, core_ids=list(range(NCORES)))

    out = np.empty((B, L, H, K, S), dtype=np.float32)
    for c in range(NCORES):
        o = res.results[c]["out"]  # (80, 65536) bf16
        out[c * BPC : (c + 1) * BPC] = (
            o.astype(np.float32).T.reshape(BPC, L, H, K, S)
        )
    return out


# revision 15
# speedup vs baseline: 12.2845x; 1.1276x over previous
"""Distributed Trainium2 kernel for nn_AncProbsLayer.

Math (reference):
    tau[b,h]  = softplus(tau_kernel[h, rate_indices[b,h]])
    R,p,Q     from tiny (H,K,20,20) kernels; Sm = D^1/2 Q D^-1/2; lam,U = eigh(Sm)
    P[b,h,k]  = D^-1/2 U diag(exp(tau*lam)) U^T D^1/2
    out       = einsum('blhz,bhkzs->blhks', inputs, P)

Device algorithm (V,W tiny host-precomputed eigen matrices; E from a
device-side indirect-DMA gather of tau_kernel + softplus + exp):
    P_comb[b]  = BDV @ (diag(E[b]) @ BDW)          (40x80, per-batch stationary)
    out[b,l,:] = in[b,l,:] @ P_comb[b]             (one matmul per batch)

HW-probed facts this version is built on (vs the previous 3-matmul
bf16 hi/lo kernel at ~150.8us/pass):
  * the inputs are EXACT one-hot vectors (0.0/1.0): an fp8_e4m3 stream
    is exact, so the hi/lo input split is unnecessary. Probing showed
    fp8-stream matmuls pipeline ~7x faster than bf16-stream ones
    (~81ns vs ~594ns per 512-col matmul), so the whole stream side
    runs fp8: 1 matmul per batch instead of 3, and half the input DMA.
  * tolerance is 2e-2 absmax-relative: a bf16 stationary (~2e-3 rel
    error) and bf16 outputs (~2e-3) are comfortably accurate -> output
    DMA is halved too (bf16 instead of fp32).
  * PSUM eviction must be DVE-ONLY whole-tile copies: mixing ACT
    (scalar) copies into the loop - as the old kernel's column-split
    eviction did - serializes the pipeline ~10x (HW-probed). A pure
    DVE eviction stream ([80,1024] fp32->bf16 per pair) runs at
    ~0.58us/pair and overlaps the matmuls cleanly. ACT's only
    main-loop job is issuing output DMAs, SP's only job input DMAs.

Distribution: data-parallel over batch B across 8 cores (128 b each);
tiny kernels + tau table replicated (no collectives needed). Streams
are fed feature-major ([40, rows]); even/odd batches live on SBUF
partitions 0-39 / 64-103 (disjoint PE row groups run concurrently).
"""

import numpy as np
import ml_dtypes

import concourse.bass as bass
import concourse.bacc as bacc
import concourse.mybir as mybir
from concourse.tile import TileContext
from concourse.masks import make_identity
from concourse.bass_utils import run_bass_kernel_spmd

# Problem constants (hardcoded per the harness contract)
B, L, H, K, S = 1024, 512, 2, 2, 20
NUM_RATES = 100000
NCORES = 8
BPC = B // NCORES          # 128 batches per core
ROWS = BPC * L             # 65536 stream rows per core
HZ = H * S                 # 40  (input feature dim)
HKS = H * K * S            # 80  (output feature dim)
CB = 32                    # batches per DMA chunk (1MB+ transfers)
F32 = mybir.dt.float32
BF16 = mybir.dt.bfloat16
F8 = mybir.dt.float8e4
NPBF16 = np.dtype(ml_dtypes.bfloat16)
NPF8 = np.dtype(mybir.dt.np(F8))

_NC_CACHE = {}


def build_nc(reps=1, timing=False):
    # reps>1 repeats the main stream inside one NEFF (benchmarking only:
    # (wall[R] - wall[1])/(R-1) cancels dispatch overhead exactly).
    # timing=True keeps the big output DRAM-internal (identical DMA work,
    # no host readback) so wall-clock deltas aren't noise-dominated.
    nc = bacc.Bacc(
        "TRN2", target_bir_lowering=False, debug=False, num_devices=NCORES
    )
    # input pre-packed on host as a 128-partition image: rows 0-39 = even-b
    # features, rows 64-103 = odd-b, rest zero. fp8 one-hot is EXACT.
    # (timing builds keep the big tensors DRAM-internal: identical DMA
    # work on garbage data, but no per-run host transfer -> low noise)
    if timing:
        in_img = nc.dram_tensor("in_img", [128, ROWS // 2], F8, kind="Internal")
        tau_tab = nc.dram_tensor("tau_tab", [H * NUM_RATES, 1], F32, kind="Internal")
    else:
        in_img = nc.declare_dram_parameter("in_img", [128, ROWS // 2], F8, isOutput=False)
        tau_tab = nc.declare_dram_parameter("tau_tab", [H * NUM_RATES, 1], F32, isOutput=False)
    offs = nc.declare_dram_parameter("offs", [BPC, H], mybir.dt.int32, isOutput=False)
    bdvT = nc.declare_dram_parameter("bdvT", [HKS, HZ], F32, isOutput=False)
    bdw = nc.declare_dram_parameter("bdw", [HKS, HKS], F32, isOutput=False)
    lam_rep = nc.declare_dram_parameter("lam_rep", [BPC, HKS], F32, isOutput=False)
    if timing:
        out = nc.dram_tensor("out", [HKS, ROWS], BF16, kind="Internal")
        outd = nc.declare_dram_parameter("outd", [128, 128], F32, isOutput=True)
    else:
        out = nc.declare_dram_parameter("out", [HKS, ROWS], BF16, isOutput=True)

    QB = 32                    # batches per pc quarter
    NQ = BPC // QB             # 4 quarters
    with TileContext(nc) as tc:
        with (
            tc.tile_pool(name="const", bufs=1) as cpool,
            tc.tile_pool(name="setup", bufs=2) as spool,
            tc.tile_pool(name="inp", bufs=3) as ipool,
            tc.tile_pool(name="ost", bufs=3) as opool,
        ):
            # ---- constants / setup ----
            bdvT_t = cpool.tile([HKS, HZ], dtype=F32)
            nc.sync.dma_start(out=bdvT_t[:], in_=bdvT[:])
            bdw_t = cpool.tile([HKS, HKS], dtype=F32)
            nc.sync.dma_start(out=bdw_t[:], in_=bdw[:])
            lam_t = cpool.tile([BPC, HKS], dtype=F32)
            nc.sync.dma_start(out=lam_t[:], in_=lam_rep[:])
            offs_t = cpool.tile([BPC, H], dtype=mybir.dt.int32)
            nc.sync.dma_start(out=offs_t[:], in_=offs[:])
            ident = cpool.tile([BPC, BPC], dtype=F32)
            make_identity(nc, ident[:])

            # ---- gather tau values: tau_raw[b,h] = tau_tab[offs[b,h]] ----
            tau_raw = cpool.tile([BPC, H], dtype=F32)
            for h in range(H):
                nc.gpsimd.indirect_dma_start(
                    out=tau_raw[:, h : h + 1],
                    out_offset=None,
                    in_=tau_tab[:],
                    in_offset=bass.IndirectOffsetOnAxis(
                        ap=offs_t[:, h : h + 1], axis=0
                    ),
                )
            # softplus(x) = ln(exp(x) + 1): the ACT table set
            # (natural_log_exp_and_others) has exp/ln/copy but no softplus.
            tau_ex = cpool.tile([BPC, H], dtype=F32)
            nc.scalar.activation(
                tau_ex[:], tau_raw[:], mybir.ActivationFunctionType.Exp
            )
            tau_sp = cpool.tile([BPC, H], dtype=F32)
            nc.scalar.activation(
                tau_sp[:], tau_ex[:], mybir.ActivationFunctionType.Ln, bias=1.0
            )

            # ---- E[b, hks] = exp(tau[b,h] * lam[hks]) ----
            E = cpool.tile([BPC, HKS], dtype=F32)
            for h in range(H):
                sl = slice(h * K * S, (h + 1) * K * S)
                nc.scalar.activation(
                    E[:, sl],
                    lam_t[:, sl],
                    mybir.ActivationFunctionType.Exp,
                    scale=tau_sp[:, h : h + 1],
                )
            # setup-only PSUM pools live in a nested scope so their banks
            # are released to the deeper main-loop PSUM pipeline
            with (
                tc.tile_pool(name="psE", bufs=1, space="PSUM") as psE,
                tc.tile_pool(name="psP", bufs=1, space="PSUM") as psP,
            ):
              # transpose E -> E_T [80, 128]: per-b columns become
              # per-partition scalars
              e_ps = psE.tile([HKS, BPC], dtype=F32, space="PSUM")
              nc.tensor.transpose(out=e_ps[:], in_=E[:], identity=ident[:])
              e_t = cpool.tile([HKS, BPC], dtype=F32)
              nc.vector.tensor_copy(out=e_t[:], in_=e_ps[:])

            # ---- setup phase: P_comb bf16 stationaries, in 4 PARITY
            # quarters (q = half*2 + b%2). Odd-parity quarters are produced
            # directly at partitions 64-103 via column tile_position on the
            # small matmuls, so no cross-partition replication is needed.
            # bdwe[:, i*80+j] = BDW[:, j] * E_T[:, b(i)] via stride-0
            # broadcast APs; P_comb = BDV @ bdwe in batched fp32 matmuls.
              e_t4 = e_t[:].rearrange("p (hh i two) -> p hh two i", two=2, i=QB)
              pc_q = []
              for q in range(NQ):
                hh, par = q // 2, q % 2
                bdwe = spool.tile([HKS, QB * HKS], dtype=F32, tag="bdwe")
                nc.gpsimd.tensor_mul(
                    bdwe[:].rearrange("p (b j) -> p b j", j=HKS),
                    bdw_t[:, None, :].to_broadcast([HKS, QB, HKS]),
                    e_t4[:, hh, par, :].to_broadcast([HKS, QB, HKS]),
                )
                pc_t = cpool.tile([128, QB * HKS], dtype=BF16, tag=f"pc{q}")
                pb = 64 * par           # partition base for this parity
                tp = (0, 64) if par else None
                for m in range((QB * HKS) // L):
                    cs = slice(m * L, (m + 1) * L)
                    pc_ps = psP.tile([128, L], dtype=F32, space="PSUM", tag="pc")
                    nc.tensor.matmul(
                        pc_ps[pb : pb + HZ, 0:L],
                        lhsT=bdvT_t[:], rhs=bdwe[:, cs],
                        start=True, stop=True, tile_position=tp,
                    )
                    nc.scalar.copy(
                        out=pc_t[pb : pb + HZ, cs], in_=pc_ps[pb : pb + HZ, :]
                    )
                pc_q.append(pc_t)

            # ---- main stream: 4 chunks x 32 batches (16 even/odd pairs) ----
            with tc.tile_pool(name="psO", bufs=4, space="PSUM") as psO:
             for _rep in range(reps):
              for ci in range(BPC // CB):
                  csl = slice(ci * (CB // 2) * L, (ci + 1) * (CB // 2) * L)
                  it = ipool.tile([128, (CB // 2) * L], dtype=F8, tag="it")
                  nc.sync.dma_start(out=it[:], in_=in_img[:, csl])
                  for jj in range(CB // 2):
                      # even/odd batch pair: even on PE rows 0-39, odd on rows
                      # 64-103 (disjoint row groups run concurrently); the two
                      # accumulation groups land in the 2 banks of one PSUM tile
                      be = ci * CB + jj * 2
                      bo = be + 1
                      qe = (be // 64) * 2 + (be % 2)
                      qo = (bo // 64) * 2 + (bo % 2)
                      bqe = (be % 64) // 2
                      bqo = (bo % 64) // 2
                      pse = slice(bqe * HKS, (bqe + 1) * HKS)
                      pso = slice(bqo * HKS, (bqo + 1) * HKS)
                      xs = slice(jj * L, (jj + 1) * L)
                      o_ps = psO.tile([HKS, 2 * L], dtype=F32, space="PSUM", tag="o")
                      oe = slice(0, L)
                      oo = slice(L, 2 * L)
                      HI = slice(64, 64 + HZ)
                      LO = slice(0, HZ)
                      nc.tensor.matmul(
                          o_ps[:, oe], lhsT=pc_q[qe][LO, pse], rhs=it[LO, xs],
                          start=True, stop=True,
                      )
                      nc.tensor.matmul(
                          o_ps[:, oo], lhsT=pc_q[qo][HI, pso], rhs=it[HI, xs],
                          start=True, stop=True,
                      )
                      # whole-tile DVE-only eviction (fp32 PSUM -> bf16 SBUF)
                      if jj % 8 == 0:
                          ot4 = opool.tile([HKS, 16 * L], dtype=BF16)
                      nc.vector.tensor_copy(
                          out=ot4[:, (jj % 8) * 2 * L : (jj % 8 + 1) * 2 * L],
                          in_=o_ps[:],
                      )
                      # out-DMA on the ACT HWDGE ring: keeps the SP ring free
                      # for input prefetch (no head-of-line blocking)
                      if jj % 8 == 7:
                          c0 = (ci * CB + (jj - 7) * 2) * L
                          nc.scalar.dma_start(
                              out=out[:, c0 : c0 + 16 * L], in_=ot4[:]
                          )
              if timing:
                  nc.sync.dma_start(out=outd[:], in_=ident[:])
    nc.finalize()
    return nc


def _host_prep(exchangeability_kernel, equilibrium_kernel):
    """Tiny (H,K,20,20) eigen prep in float64 on host -> BDV, BDW, lam."""
    ek = exchangeability_kernel.astype(np.float64)
    eq = equilibrium_kernel.astype(np.float64)
    Rm = 0.5 * (ek + np.swapaxes(ek, -1, -2))
    Rm = np.logaddexp(0.0, Rm)  # softplus
    Rm = Rm * (1.0 - np.eye(S))
    # softmax
    em = eq - eq.max(axis=-1, keepdims=True)
    p = np.exp(em)
    p /= p.sum(axis=-1, keepdims=True)
    Q = Rm * p[..., None, :]
    row = Q.sum(axis=-1)
    Q = Q - row[..., :, None] * np.eye(S)
    mue = (p * row).sum(axis=-1)[..., None, None]
    Q = Q / np.maximum(mue, 1e-16)
    sqrt_p = np.sqrt(p)
    inv_sqrt_p = 1.0 / sqrt_p
    Sm = sqrt_p[..., :, None] * Q * inv_sqrt_p[..., None, :]
    Sm = 0.5 * (Sm + np.swapaxes(Sm, -1, -2))
    lam, U = np.linalg.eigh(Sm)  # (H,K,S), (H,K,S,S)

    BDV = np.zeros((HZ, HKS), dtype=np.float64)
    BDW = np.zeros((HKS, HKS), dtype=np.float64)
    for h in range(H):
        for k in range(K):
            c = h * K * S + k * S
            # V[z,s] = U[z,s]/sqrt(p[z]) ; rows = (h,z), cols = (h,k,s)
            BDV[h * S : (h + 1) * S, c : c + S] = inv_sqrt_p[h, k][:, None] * U[h, k]
            # BDW[(h,k,s),(h,k,j)] = sqrt(p[j]) * U[j,s]
            BDW[c : c + S, c : c + S] = (sqrt_p[h, k][:, None] * U[h, k]).T
    lam_flat = lam.reshape(HKS)
    return BDV.astype(np.float32), BDW.astype(np.float32), lam_flat.astype(np.float32)


def kernel(inputs, rate_indices, tau_kernel, exchangeability_kernel, equilibrium_kernel):
    inputs = np.asarray(inputs, dtype=np.float32)
    rate_indices = np.asarray(rate_indices)
    tau_kernel = np.asarray(tau_kernel, dtype=np.float32)

    BDV, BDW, lam_flat = _host_prep(
        np.asarray(exchangeability_kernel), np.asarray(equilibrium_kernel)
    )
    BDV_T = np.ascontiguousarray(BDV.T)
    lam_rep = np.broadcast_to(lam_flat, (BPC, HKS)).copy()
    tau_tab = tau_kernel.reshape(H * NUM_RATES, 1)

    if "nc" not in _NC_CACHE:
        _NC_CACHE["nc"] = build_nc()
    nc = _NC_CACHE["nc"]

    in_maps = []
    for c in range(NCORES):
        bsl = slice(c * BPC, (c + 1) * BPC)
        # feature-major stream layout: [40, 65536]; the one-hot values are
        # exactly representable in fp8_e4m3 (0.0 / 1.0), no precision loss
        inT_c = np.ascontiguousarray(inputs[bsl].reshape(BPC * L, HZ).T)
        f8 = inT_c.astype(NPF8)
        f83 = f8.reshape(HZ, BPC, L)
        img = np.zeros((128, ROWS // 2), dtype=NPF8)
        img[:HZ] = f83[:, 0::2].reshape(HZ, ROWS // 2)
        img[64 : 64 + HZ] = f83[:, 1::2].reshape(HZ, ROWS // 2)
        offs_c = (
            np.arange(H, dtype=np.int64)[None, :] * NUM_RATES
            + rate_indices[bsl].astype(np.int64)
        ).astype(np.int32)
        in_maps.append(
            {
                "in_img": img,
                "tau_tab": tau_tab,
                "offs": np.ascontiguousarray(offs_c),
                "bdvT": BDV_T,
                "bdw": BDW,
                "lam_rep": lam_rep,
            }
        )

    _NC_CACHE["in_maps"] = in_maps
    res = run_bass_kernel_spmd(nc, in_maps, core_ids=list(range(NCORES)))

    out = np.empty((B, L, H, K, S), dtype=np.float32)
    for c in range(NCORES):
        o = res.results[c]["out"]  # (80, 65536) bf16
        out[c * BPC : (c + 1) * BPC] = (
            o.astype(np.float32).T.reshape(BPC, L, H, K, S)
        )
    return out


# revision 17
# speedup vs baseline: 13.3241x; 1.0846x over previous
"""Distributed Trainium2 kernel for nn_AncProbsLayer.

Math (reference):
    tau[b,h]  = softplus(tau_kernel[h, rate_indices[b,h]])
    R,p,Q     from tiny (H,K,20,20) kernels; Sm = D^1/2 Q D^-1/2; lam,U = eigh(Sm)
    P[b,h,k]  = D^-1/2 U diag(exp(tau*lam)) U^T D^1/2
    out       = einsum('blhz,bhkzs->blhks', inputs, P)

Device algorithm (V,W tiny host-precomputed eigen matrices; E from a
device-side indirect-DMA gather of tau_kernel + softplus + exp):
    P_comb[b]  = BDV @ (diag(E[b]) @ BDW)          (40x80, per-batch stationary)
    out[b,l,:] = in[b,l,:] @ P_comb[b]             (one matmul per batch)

HW-probed facts this version is built on (vs the previous 3-matmul
bf16 hi/lo kernel at ~150.8us/pass):
  * the inputs are EXACT one-hot vectors (0.0/1.0): an fp8_e4m3 stream
    is exact, so the hi/lo input split is unnecessary. Probing showed
    fp8-stream matmuls pipeline ~7x faster than bf16-stream ones
    (~81ns vs ~594ns per 512-col matmul), so the whole stream side
    runs fp8: 1 matmul per batch instead of 3, and half the input DMA.
  * tolerance is 2e-2 absmax-relative: a bf16 stationary (~2e-3 rel
    error) and bf16 outputs (~2e-3) are comfortably accurate -> output
    DMA is halved too (bf16 instead of fp32).
  * PSUM eviction must be DVE-ONLY whole-tile copies: mixing ACT
    (scalar) copies into the loop - as the old kernel's column-split
    eviction did - serializes the pipeline ~10x (HW-probed). A pure
    DVE eviction stream ([80,1024] fp32->bf16 per pair) runs at
    ~0.58us/pair and overlaps the matmuls cleanly. ACT's only
    main-loop job is issuing output DMAs, SP's only job input DMAs.

Distribution: data-parallel over batch B across 8 cores (128 b each);
tiny kernels + tau table replicated (no collectives needed). Streams
are fed feature-major ([40, rows]); even/odd batches live on SBUF
partitions 0-39 / 64-103 (disjoint PE row groups run concurrently).
"""

import numpy as np
import ml_dtypes

import concourse.bass as bass
import concourse.bacc as bacc
import concourse.mybir as mybir
from concourse.tile import TileContext
from concourse.masks import make_identity
from concourse.bass_utils import run_bass_kernel_spmd

# Problem constants (hardcoded per the harness contract)
B, L, H, K, S = 1024, 512, 2, 2, 20
NUM_RATES = 100000
NCORES = 8
BPC = B // NCORES          # 128 batches per core
ROWS = BPC * L             # 65536 stream rows per core
HZ = H * S                 # 40  (input feature dim)
HKS = H * K * S            # 80  (output feature dim)
CB = 64                    # batches per DMA chunk (2MB+ input transfers)
F32 = mybir.dt.float32
BF16 = mybir.dt.bfloat16
F8 = mybir.dt.float8e4
NPBF16 = np.dtype(ml_dtypes.bfloat16)
NPF8 = np.dtype(mybir.dt.np(F8))

_NC_CACHE = {}


def build_nc(reps=1, timing=False):
    # reps>1 repeats the main stream inside one NEFF (benchmarking only:
    # (wall[R] - wall[1])/(R-1) cancels dispatch overhead exactly).
    # timing=True keeps the big output DRAM-internal (identical DMA work,
    # no host readback) so wall-clock deltas aren't noise-dominated.
    nc = bacc.Bacc(
        "TRN2", target_bir_lowering=False, debug=False, num_devices=NCORES
    )
    # input pre-packed on host as a 128-partition image: rows 0-39 = even-b
    # features, rows 64-103 = odd-b, rest zero. fp8 one-hot is EXACT.
    # (timing builds keep the big tensors DRAM-internal: identical DMA
    # work on garbage data, but no per-run host transfer -> low noise)
    if timing:
        in_img = nc.dram_tensor("in_img", [128, ROWS // 2], F8, kind="Internal")
        tau_tab = nc.dram_tensor("tau_tab", [H * NUM_RATES, 1], F32, kind="Internal")
    else:
        in_img = nc.declare_dram_parameter("in_img", [128, ROWS // 2], F8, isOutput=False)
        tau_tab = nc.declare_dram_parameter("tau_tab", [H * NUM_RATES, 1], F32, isOutput=False)
    offs = nc.declare_dram_parameter("offs", [BPC, H], mybir.dt.int32, isOutput=False)
    bdvT = nc.declare_dram_parameter("bdvT", [HKS, HZ], F32, isOutput=False)
    bdw = nc.declare_dram_parameter("bdw", [HKS, HKS], F32, isOutput=False)
    lam_rep = nc.declare_dram_parameter("lam_rep", [BPC, HKS], F32, isOutput=False)
    if timing:
        out = nc.dram_tensor("out", [HKS, ROWS], BF16, kind="Internal")
        outd = nc.declare_dram_parameter("outd", [128, 128], F32, isOutput=True)
    else:
        out = nc.declare_dram_parameter("out", [HKS, ROWS], BF16, isOutput=True)

    QB = 32                    # batches per pc quarter
    NQ = BPC // QB             # 4 quarters
    with TileContext(nc) as tc:
        with (
            tc.tile_pool(name="const", bufs=1) as cpool,
            tc.tile_pool(name="setup", bufs=2) as spool,
            tc.tile_pool(name="inp", bufs=2) as ipool,
            tc.tile_pool(name="ost", bufs=3) as opool,
        ):
            # ---- constants / setup ----
            bdvT_t = cpool.tile([HKS, HZ], dtype=F32)
            nc.sync.dma_start(out=bdvT_t[:], in_=bdvT[:])
            bdw_t = cpool.tile([HKS, HKS], dtype=F32)
            nc.sync.dma_start(out=bdw_t[:], in_=bdw[:])
            lam_t = cpool.tile([BPC, HKS], dtype=F32)
            nc.sync.dma_start(out=lam_t[:], in_=lam_rep[:])
            offs_t = cpool.tile([BPC, H], dtype=mybir.dt.int32)
            nc.sync.dma_start(out=offs_t[:], in_=offs[:])
            ident = cpool.tile([BPC, BPC], dtype=F32)
            make_identity(nc, ident[:])

            # ---- gather tau values: tau_raw[b,h] = tau_tab[offs[b,h]] ----
            tau_raw = cpool.tile([BPC, H], dtype=F32)
            for h in range(H):
                nc.gpsimd.indirect_dma_start(
                    out=tau_raw[:, h : h + 1],
                    out_offset=None,
                    in_=tau_tab[:],
                    in_offset=bass.IndirectOffsetOnAxis(
                        ap=offs_t[:, h : h + 1], axis=0
                    ),
                )
            # softplus(x) = ln(exp(x) + 1): the ACT table set
            # (natural_log_exp_and_others) has exp/ln/copy but no softplus.
            tau_ex = cpool.tile([BPC, H], dtype=F32)
            nc.scalar.activation(
                tau_ex[:], tau_raw[:], mybir.ActivationFunctionType.Exp
            )
            tau_sp = cpool.tile([BPC, H], dtype=F32)
            nc.scalar.activation(
                tau_sp[:], tau_ex[:], mybir.ActivationFunctionType.Ln, bias=1.0
            )

            # ---- E[b, hks] = exp(tau[b,h] * lam[hks]) ----
            E = cpool.tile([BPC, HKS], dtype=F32)
            for h in range(H):
                sl = slice(h * K * S, (h + 1) * K * S)
                nc.scalar.activation(
                    E[:, sl],
                    lam_t[:, sl],
                    mybir.ActivationFunctionType.Exp,
                    scale=tau_sp[:, h : h + 1],
                )
            # setup-only PSUM pools live in a nested scope so their banks
            # are released to the deeper main-loop PSUM pipeline
            with (
                tc.tile_pool(name="psE", bufs=1, space="PSUM") as psE,
                tc.tile_pool(name="psP", bufs=1, space="PSUM") as psP,
            ):
              # transpose E -> E_T [80, 128]: per-b columns become
              # per-partition scalars
              e_ps = psE.tile([HKS, BPC], dtype=F32, space="PSUM")
              nc.tensor.transpose(out=e_ps[:], in_=E[:], identity=ident[:])
              e_t = cpool.tile([HKS, BPC], dtype=F32)
              nc.vector.tensor_copy(out=e_t[:], in_=e_ps[:])

            # ---- setup phase: P_comb bf16 stationaries, in 4 PARITY
            # quarters (q = half*2 + b%2). Odd-parity quarters are produced
            # directly at partitions 64-103 via column tile_position on the
            # small matmuls, so no cross-partition replication is needed.
            # bdwe[:, i*80+j] = BDW[:, j] * E_T[:, b(i)] via stride-0
            # broadcast APs; P_comb = BDV @ bdwe in batched fp32 matmuls.
              e_t4 = e_t[:].rearrange("p (hh i two) -> p hh two i", two=2, i=QB)
              pc_q = []
              for q in range(NQ):
                hh, par = q // 2, q % 2
                bdwe = spool.tile([HKS, QB * HKS], dtype=F32, tag="bdwe")
                nc.gpsimd.tensor_mul(
                    bdwe[:].rearrange("p (b j) -> p b j", j=HKS),
                    bdw_t[:, None, :].to_broadcast([HKS, QB, HKS]),
                    e_t4[:, hh, par, :].to_broadcast([HKS, QB, HKS]),
                )
                pc_t = cpool.tile([128, QB * HKS], dtype=BF16, tag=f"pc{q}")
                pb = 64 * par           # partition base for this parity
                tp = (0, 64) if par else None
                for m in range((QB * HKS) // L):
                    cs = slice(m * L, (m + 1) * L)
                    pc_ps = psP.tile([128, L], dtype=F32, space="PSUM", tag="pc")
                    nc.tensor.matmul(
                        pc_ps[pb : pb + HZ, 0:L],
                        lhsT=bdvT_t[:], rhs=bdwe[:, cs],
                        start=True, stop=True, tile_position=tp,
                    )
                    nc.scalar.copy(
                        out=pc_t[pb : pb + HZ, cs], in_=pc_ps[pb : pb + HZ, :]
                    )
                pc_q.append(pc_t)

            # ---- main stream: 4 chunks x 32 batches (16 even/odd pairs) ----
            with tc.tile_pool(name="psO", bufs=4, space="PSUM") as psO:
             for _rep in range(reps):
              for ci in range(BPC // CB):
                  csl = slice(ci * (CB // 2) * L, (ci + 1) * (CB // 2) * L)
                  it = ipool.tile([128, (CB // 2) * L], dtype=F8, tag="it")
                  nc.sync.dma_start(out=it[:], in_=in_img[:, csl])
                  for jj in range(CB // 2):
                      # even/odd batch pair: even on PE rows 0-39, odd on rows
                      # 64-103 (disjoint row groups run concurrently); the two
                      # accumulation groups land in the 2 banks of one PSUM tile
                      be = ci * CB + jj * 2
                      bo = be + 1
                      qe = (be // 64) * 2 + (be % 2)
                      qo = (bo // 64) * 2 + (bo % 2)
                      bqe = (be % 64) // 2
                      bqo = (bo % 64) // 2
                      pse = slice(bqe * HKS, (bqe + 1) * HKS)
                      pso = slice(bqo * HKS, (bqo + 1) * HKS)
                      xs = slice(jj * L, (jj + 1) * L)
                      o_ps = psO.tile([HKS, 2 * L], dtype=F32, space="PSUM", tag="o")
                      oe = slice(0, L)
                      oo = slice(L, 2 * L)
                      HI = slice(64, 64 + HZ)
                      LO = slice(0, HZ)
                      nc.tensor.matmul(
                          o_ps[:, oe], lhsT=pc_q[qe][LO, pse], rhs=it[LO, xs],
                          start=True, stop=True,
                      )
                      nc.tensor.matmul(
                          o_ps[:, oo], lhsT=pc_q[qo][HI, pso], rhs=it[HI, xs],
                          start=True, stop=True,
                      )
                      # whole-tile DVE-only eviction (fp32 PSUM -> bf16 SBUF)
                      if jj % 8 == 0:
                          ot4 = opool.tile([HKS, 16 * L], dtype=BF16)
                      nc.vector.tensor_copy(
                          out=ot4[:, (jj % 8) * 2 * L : (jj % 8 + 1) * 2 * L],
                          in_=o_ps[:],
                      )
                      # out-DMA on the ACT HWDGE ring: keeps the SP ring free
                      # for input prefetch (no head-of-line blocking)
                      if jj % 8 == 7:
                          c0 = (ci * CB + (jj - 7) * 2) * L
                          nc.scalar.dma_start(
                              out=out[:, c0 : c0 + 16 * L], in_=ot4[:]
                          )
              if timing:
                  nc.sync.dma_start(out=outd[:], in_=ident[:])
    nc.finalize()
    return nc


def _host_prep(exchangeability_kernel, equilibrium_kernel):
    """Tiny (H,K,20,20) eigen prep in float64 on host -> BDV, BDW, lam."""
    ek = exchangeability_kernel.astype(np.float64)
    eq = equilibrium_kernel.astype(np.float64)
    Rm = 0.5 * (ek + np.swapaxes(ek, -1, -2))
    Rm = np.logaddexp(0.0, Rm)  # softplus
    Rm = Rm * (1.0 - np.eye(S))
    # softmax
    em = eq - eq.max(axis=-1, keepdims=True)
    p = np.exp(em)
    p /= p.sum(axis=-1, keepdims=True)
    Q = Rm * p[..., None, :]
    row = Q.sum(axis=-1)
    Q = Q - row[..., :, None] * np.eye(S)
    mue = (p * row).sum(axis=-1)[..., None, None]
    Q = Q / np.maximum(mue, 1e-16)
    sqrt_p = np.sqrt(p)
    inv_sqrt_p = 1.0 / sqrt_p
    Sm = sqrt_p[..., :, None] * Q * inv_sqrt_p[..., None, :]
    Sm = 0.5 * (Sm + np.swapaxes(Sm, -1, -2))
    lam, U = np.linalg.eigh(Sm)  # (H,K,S), (H,K,S,S)

    BDV = np.zeros((HZ, HKS), dtype=np.float64)
    BDW = np.zeros((HKS, HKS), dtype=np.float64)
    for h in range(H):
        for k in range(K):
            c = h * K * S + k * S
            # V[z,s] = U[z,s]/sqrt(p[z]) ; rows = (h,z), cols = (h,k,s)
            BDV[h * S : (h + 1) * S, c : c + S] = inv_sqrt_p[h, k][:, None] * U[h, k]
            # BDW[(h,k,s),(h,k,j)] = sqrt(p[j]) * U[j,s]
            BDW[c : c + S, c : c + S] = (sqrt_p[h, k][:, None] * U[h, k]).T
    lam_flat = lam.reshape(HKS)
    return BDV.astype(np.float32), BDW.astype(np.float32), lam_flat.astype(np.float32)


def kernel(inputs, rate_indices, tau_kernel, exchangeability_kernel, equilibrium_kernel):
    inputs = np.asarray(inputs, dtype=np.float32)
    rate_indices = np.asarray(rate_indices)
    tau_kernel = np.asarray(tau_kernel, dtype=np.float32)

    BDV, BDW, lam_flat = _host_prep(
        np.asarray(exchangeability_kernel), np.asarray(equilibrium_kernel)
    )
    BDV_T = np.ascontiguousarray(BDV.T)
    lam_rep = np.broadcast_to(lam_flat, (BPC, HKS)).copy()
    tau_tab = tau_kernel.reshape(H * NUM_RATES, 1)

    if "nc" not in _NC_CACHE:
        _NC_CACHE["nc"] = build_nc()
    nc = _NC_CACHE["nc"]

    in_maps = []
    for c in range(NCORES):
        bsl = slice(c * BPC, (c + 1) * BPC)
        # feature-major stream layout: [40, 65536]; the one-hot values are
        # exactly representable in fp8_e4m3 (0.0 / 1.0), no precision loss
        inT_c = np.ascontiguousarray(inputs[bsl].reshape(BPC * L, HZ).T)
        f8 = inT_c.astype(NPF8)
        f83 = f8.reshape(HZ, BPC, L)
        img = np.zeros((128, ROWS // 2), dtype=NPF8)
        img[:HZ] = f83[:, 0::2].reshape(HZ, ROWS // 2)
        img[64 : 64 + HZ] = f83[:, 1::2].reshape(HZ, ROWS // 2)
        offs_c = (
            np.arange(H, dtype=np.int64)[None, :] * NUM_RATES
            + rate_indices[bsl].astype(np.int64)
        ).astype(np.int32)
        in_maps.append(
            {
                "in_img": img,
                "tau_tab": tau_tab,
                "offs": np.ascontiguousarray(offs_c),
                "bdvT": BDV_T,
                "bdw": BDW,
                "lam_rep": lam_rep,
            }
        )

    _NC_CACHE["in_maps"] = in_maps
    res = run_bass_kernel_spmd(nc, in_maps, core_ids=list(range(NCORES)))

    out = np.empty((B, L, H, K, S), dtype=np.float32)
    for c in range(NCORES):
        o = res.results[c]["out"]  # (80, 65536) bf16
        out[c * BPC : (c + 1) * BPC] = (
            o.astype(np.float32).T.reshape(BPC, L, H, K, S)
        )
    return out


# revision 19
# speedup vs baseline: 14.3632x; 1.0780x over previous
"""Distributed Trainium2 kernel for nn_AncProbsLayer.

Math (reference):
    tau[b,h]  = softplus(tau_kernel[h, rate_indices[b,h]])
    R,p,Q     from tiny (H,K,20,20) kernels; Sm = D^1/2 Q D^-1/2; lam,U = eigh(Sm)
    P[b,h,k]  = D^-1/2 U diag(exp(tau*lam)) U^T D^1/2
    out       = einsum('blhz,bhkzs->blhks', inputs, P)

Device algorithm (V,W tiny host-precomputed eigen matrices; E from a
device-side indirect-DMA gather of tau_kernel + softplus + exp):
    P_comb[b]  = BDV @ (diag(E[b]) @ BDW)          (40x80, per-batch stationary)
    out[b,l,:] = in[b,l,:] @ P_comb[b]             (one matmul per batch)

HW-probed facts this version is built on (vs the previous 3-matmul
bf16 hi/lo kernel at ~150.8us/pass):
  * the inputs are EXACT one-hot vectors (0.0/1.0): an fp8_e4m3 stream
    is exact, so the hi/lo input split is unnecessary. Probing showed
    fp8-stream matmuls pipeline ~7x faster than bf16-stream ones
    (~81ns vs ~594ns per 512-col matmul), so the whole stream side
    runs fp8: 1 matmul per batch instead of 3, and half the input DMA.
  * tolerance is 2e-2 absmax-relative: a bf16 stationary (~2e-3 rel
    error) and bf16 outputs (~2e-3) are comfortably accurate -> output
    DMA is halved too (bf16 instead of fp32).
  * PSUM eviction must be DVE-ONLY whole-tile copies: mixing ACT
    (scalar) copies into the loop - as the old kernel's column-split
    eviction did - serializes the pipeline ~10x (HW-probed). A pure
    DVE eviction stream ([80,1024] fp32->bf16 per pair) runs at
    ~0.58us/pair and overlaps the matmuls cleanly. ACT's only
    main-loop job is issuing output DMAs, SP's only job input DMAs.

Distribution: data-parallel over batch B across 8 cores (128 b each);
tiny kernels + tau table replicated (no collectives needed). Streams
are fed feature-major ([40, rows]); even/odd batches live on SBUF
partitions 0-39 / 64-103 (disjoint PE row groups run concurrently).
"""

import numpy as np
import ml_dtypes

import concourse.bass as bass
import concourse.bacc as bacc
import concourse.mybir as mybir
from concourse.tile import TileContext
from concourse.masks import make_identity
from concourse.bass_utils import run_bass_kernel_spmd

# Problem constants (hardcoded per the harness contract)
B, L, H, K, S = 1024, 512, 2, 2, 20
NUM_RATES = 100000
NCORES = 8
BPC = B // NCORES          # 128 batches per core
ROWS = BPC * L             # 65536 stream rows per core
HZ = H * S                 # 40  (input feature dim)
HKS = H * K * S            # 80  (output feature dim)
CB = 64                    # batches per DMA chunk (2MB+ input transfers)
F32 = mybir.dt.float32
BF16 = mybir.dt.bfloat16
F8 = mybir.dt.float8e4
NPBF16 = np.dtype(ml_dtypes.bfloat16)
NPF8 = np.dtype(mybir.dt.np(F8))

_NC_CACHE = {}


def build_nc(reps=1, timing=False):
    # reps>1 repeats the main stream inside one NEFF (benchmarking only:
    # (wall[R] - wall[1])/(R-1) cancels dispatch overhead exactly).
    # timing=True keeps the big output DRAM-internal (identical DMA work,
    # no host readback) so wall-clock deltas aren't noise-dominated.
    nc = bacc.Bacc(
        "TRN2", target_bir_lowering=False, debug=False, num_devices=NCORES
    )
    # input pre-packed on host as a 128-partition image: rows 0-39 = even-b
    # features, rows 64-103 = odd-b, rest zero. fp8 one-hot is EXACT.
    # (timing builds keep the big tensors DRAM-internal: identical DMA
    # work on garbage data, but no per-run host transfer -> low noise)
    if timing:
        in_img = nc.dram_tensor("in_img", [128, ROWS // 2], F8, kind="Internal")
        tau_tab = nc.dram_tensor("tau_tab", [H * NUM_RATES, 1], F32, kind="Internal")
    else:
        in_img = nc.declare_dram_parameter("in_img", [128, ROWS // 2], F8, isOutput=False)
        tau_tab = nc.declare_dram_parameter("tau_tab", [H * NUM_RATES, 1], F32, isOutput=False)
    offs = nc.declare_dram_parameter("offs", [BPC, H], mybir.dt.int32, isOutput=False)
    bdvT = nc.declare_dram_parameter("bdvT", [HKS, HZ], F32, isOutput=False)
    bdw = nc.declare_dram_parameter("bdw", [HKS, HKS], F32, isOutput=False)
    lam_rep = nc.declare_dram_parameter("lam_rep", [BPC, HKS], F32, isOutput=False)
    if timing:
        out = nc.dram_tensor("out", [HKS, ROWS], BF16, kind="Internal")
        outd = nc.declare_dram_parameter("outd", [128, 128], F32, isOutput=True)
    else:
        out = nc.declare_dram_parameter("out", [HKS, ROWS], BF16, isOutput=True)

    QB = 32                    # batches per pc quarter
    NQ = BPC // QB             # 4 quarters
    with TileContext(nc) as tc:
        with (
            tc.tile_pool(name="const", bufs=1) as cpool,
            tc.tile_pool(name="setup", bufs=2) as spool,
            tc.tile_pool(name="inp", bufs=2) as ipool,
            tc.tile_pool(name="ost", bufs=2) as opool,
        ):
            # ---- constants / setup ----
            bdvT_t = cpool.tile([HKS, HZ], dtype=F32)
            nc.sync.dma_start(out=bdvT_t[:], in_=bdvT[:])
            bdw_t = cpool.tile([HKS, HKS], dtype=F32)
            nc.sync.dma_start(out=bdw_t[:], in_=bdw[:])
            lam_t = cpool.tile([BPC, HKS], dtype=F32)
            nc.sync.dma_start(out=lam_t[:], in_=lam_rep[:])
            offs_t = cpool.tile([BPC, H], dtype=mybir.dt.int32)
            nc.sync.dma_start(out=offs_t[:], in_=offs[:])
            ident = cpool.tile([BPC, BPC], dtype=F32)
            make_identity(nc, ident[:])

            # ---- gather tau values: tau_raw[b,h] = tau_tab[offs[b,h]] ----
            tau_raw = cpool.tile([BPC, H], dtype=F32)
            for h in range(H):
                nc.gpsimd.indirect_dma_start(
                    out=tau_raw[:, h : h + 1],
                    out_offset=None,
                    in_=tau_tab[:],
                    in_offset=bass.IndirectOffsetOnAxis(
                        ap=offs_t[:, h : h + 1], axis=0
                    ),
                )
            # softplus(x) = ln(exp(x) + 1): the ACT table set
            # (natural_log_exp_and_others) has exp/ln/copy but no softplus.
            tau_ex = cpool.tile([BPC, H], dtype=F32)
            nc.scalar.activation(
                tau_ex[:], tau_raw[:], mybir.ActivationFunctionType.Exp
            )
            tau_sp = cpool.tile([BPC, H], dtype=F32)
            nc.scalar.activation(
                tau_sp[:], tau_ex[:], mybir.ActivationFunctionType.Ln, bias=1.0
            )

            # ---- E[b, hks] = exp(tau[b,h] * lam[hks]) ----
            E = cpool.tile([BPC, HKS], dtype=F32)
            for h in range(H):
                sl = slice(h * K * S, (h + 1) * K * S)
                nc.scalar.activation(
                    E[:, sl],
                    lam_t[:, sl],
                    mybir.ActivationFunctionType.Exp,
                    scale=tau_sp[:, h : h + 1],
                )
            # setup-only PSUM pools live in a nested scope so their banks
            # are released to the deeper main-loop PSUM pipeline
            with (
                tc.tile_pool(name="psE", bufs=1, space="PSUM") as psE,
                tc.tile_pool(name="psP", bufs=1, space="PSUM") as psP,
            ):
              # transpose E -> E_T [80, 128]: per-b columns become
              # per-partition scalars
              e_ps = psE.tile([HKS, BPC], dtype=F32, space="PSUM")
              nc.tensor.transpose(out=e_ps[:], in_=E[:], identity=ident[:])
              e_t = cpool.tile([HKS, BPC], dtype=F32)
              nc.vector.tensor_copy(out=e_t[:], in_=e_ps[:])

            # ---- setup phase: P_comb bf16 stationaries, in 4 PARITY
            # quarters (q = half*2 + b%2). Odd-parity quarters are produced
            # directly at partitions 64-103 via column tile_position on the
            # small matmuls, so no cross-partition replication is needed.
            # bdwe[:, i*80+j] = BDW[:, j] * E_T[:, b(i)] via stride-0
            # broadcast APs; P_comb = BDV @ bdwe in batched fp32 matmuls.
              e_t4 = e_t[:].rearrange("p (hh i two) -> p hh two i", two=2, i=QB)
              pc_q = []
              for q in range(NQ):
                hh, par = q // 2, q % 2
                bdwe = spool.tile([HKS, QB * HKS], dtype=F32, tag="bdwe")
                nc.gpsimd.tensor_mul(
                    bdwe[:].rearrange("p (b j) -> p b j", j=HKS),
                    bdw_t[:, None, :].to_broadcast([HKS, QB, HKS]),
                    e_t4[:, hh, par, :].to_broadcast([HKS, QB, HKS]),
                )
                pc_t = cpool.tile([128, QB * HKS], dtype=BF16, tag=f"pc{q}")
                pb = 64 * par           # partition base for this parity
                tp = (0, 64) if par else None
                for m in range((QB * HKS) // L):
                    cs = slice(m * L, (m + 1) * L)
                    pc_ps = psP.tile([128, L], dtype=F32, space="PSUM", tag="pc")
                    nc.tensor.matmul(
                        pc_ps[pb : pb + HZ, 0:L],
                        lhsT=bdvT_t[:], rhs=bdwe[:, cs],
                        start=True, stop=True, tile_position=tp,
                    )
                    nc.scalar.copy(
                        out=pc_t[pb : pb + HZ, cs], in_=pc_ps[pb : pb + HZ, :]
                    )
                pc_q.append(pc_t)

            # ---- main stream: 4 chunks x 32 batches (16 even/odd pairs) ----
            with tc.tile_pool(name="psO", bufs=4, space="PSUM") as psO:
             for _rep in range(reps):
              for ci in range(BPC // CB):
                  csl = slice(ci * (CB // 2) * L, (ci + 1) * (CB // 2) * L)
                  it = ipool.tile([128, (CB // 2) * L], dtype=F8, tag="it")
                  nc.sync.dma_start(out=it[:], in_=in_img[:, csl])
                  for jj in range(CB // 2):
                      # even/odd batch pair: even on PE rows 0-39, odd on rows
                      # 64-103 (disjoint row groups run concurrently); the two
                      # accumulation groups land in the 2 banks of one PSUM tile
                      be = ci * CB + jj * 2
                      bo = be + 1
                      qe = (be // 64) * 2 + (be % 2)
                      qo = (bo // 64) * 2 + (bo % 2)
                      bqe = (be % 64) // 2
                      bqo = (bo % 64) // 2
                      pse = slice(bqe * HKS, (bqe + 1) * HKS)
                      pso = slice(bqo * HKS, (bqo + 1) * HKS)
                      xs = slice(jj * L, (jj + 1) * L)
                      o_ps = psO.tile([HKS, 2 * L], dtype=F32, space="PSUM", tag="o")
                      oe = slice(0, L)
                      oo = slice(L, 2 * L)
                      HI = slice(64, 64 + HZ)
                      LO = slice(0, HZ)
                      nc.tensor.matmul(
                          o_ps[:, oe], lhsT=pc_q[qe][LO, pse], rhs=it[LO, xs],
                          start=True, stop=True,
                      )
                      nc.tensor.matmul(
                          o_ps[:, oo], lhsT=pc_q[qo][HI, pso], rhs=it[HI, xs],
                          start=True, stop=True,
                      )
                      # whole-tile DVE-only eviction (fp32 PSUM -> bf16 SBUF)
                      if jj % 16 == 0:
                          ot4 = opool.tile([HKS, 32 * L], dtype=BF16)
                      nc.vector.tensor_copy(
                          out=ot4[:, (jj % 16) * 2 * L : (jj % 16 + 1) * 2 * L],
                          in_=o_ps[:],
                      )
                      # 2.6MB out-DMAs, alternated across BOTH HWDGE rings
                      # (ACT and SP - SP is idle after the 2 input prefetches)
                      if jj % 16 == 15:
                          c0 = (ci * CB + (jj - 15) * 2) * L
                          eng = nc.scalar if (ci * 2 + jj // 16) % 2 == 0 else nc.sync
                          eng.dma_start(
                              out=out[:, c0 : c0 + 32 * L], in_=ot4[:]
                          )
              if timing:
                  nc.sync.dma_start(out=outd[:], in_=ident[:])
    nc.finalize()
    return nc


def _host_prep(exchangeability_kernel, equilibrium_kernel):
    """Tiny (H,K,20,20) eigen prep in float64 on host -> BDV, BDW, lam."""
    ek = exchangeability_kernel.astype(np.float64)
    eq = equilibrium_kernel.astype(np.float64)
    Rm = 0.5 * (ek + np.swapaxes(ek, -1, -2))
    Rm = np.logaddexp(0.0, Rm)  # softplus
    Rm = Rm * (1.0 - np.eye(S))
    # softmax
    em = eq - eq.max(axis=-1, keepdims=True)
    p = np.exp(em)
    p /= p.sum(axis=-1, keepdims=True)
    Q = Rm * p[..., None, :]
    row = Q.sum(axis=-1)
    Q = Q - row[..., :, None] * np.eye(S)
    mue = (p * row).sum(axis=-1)[..., None, None]
    Q = Q / np.maximum(mue, 1e-16)
    sqrt_p = np.sqrt(p)
    inv_sqrt_p = 1.0 / sqrt_p
    Sm = sqrt_p[..., :, None] * Q * inv_sqrt_p[..., None, :]
    Sm = 0.5 * (Sm + np.swapaxes(Sm, -1, -2))
    lam, U = np.linalg.eigh(Sm)  # (H,K,S), (H,K,S,S)

    BDV = np.zeros((HZ, HKS), dtype=np.float64)
    BDW = np.zeros((HKS, HKS), dtype=np.float64)
    for h in range(H):
        for k in range(K):
            c = h * K * S + k * S
            # V[z,s] = U[z,s]/sqrt(p[z]) ; rows = (h,z), cols = (h,k,s)
            BDV[h * S : (h + 1) * S, c : c + S] = inv_sqrt_p[h, k][:, None] * U[h, k]
            # BDW[(h,k,s),(h,k,j)] = sqrt(p[j]) * U[j,s]
            BDW[c : c + S, c : c + S] = (sqrt_p[h, k][:, None] * U[h, k]).T
    lam_flat = lam.reshape(HKS)
    return BDV.astype(np.float32), BDW.astype(np.float32), lam_flat.astype(np.float32)


def kernel(inputs, rate_indices, tau_kernel, exchangeability_kernel, equilibrium_kernel):
    inputs = np.asarray(inputs, dtype=np.float32)
    rate_indices = np.asarray(rate_indices)
    tau_kernel = np.asarray(tau_kernel, dtype=np.float32)

    BDV, BDW, lam_flat = _host_prep(
        np.asarray(exchangeability_kernel), np.asarray(equilibrium_kernel)
    )
    BDV_T = np.ascontiguousarray(BDV.T)
    lam_rep = np.broadcast_to(lam_flat, (BPC, HKS)).copy()
    tau_tab = tau_kernel.reshape(H * NUM_RATES, 1)

    if "nc" not in _NC_CACHE:
        _NC_CACHE["nc"] = build_nc()
    nc = _NC_CACHE["nc"]

    in_maps = []
    for c in range(NCORES):
        bsl = slice(c * BPC, (c + 1) * BPC)
        # feature-major stream layout: [40, 65536]; the one-hot values are
        # exactly representable in fp8_e4m3 (0.0 / 1.0), no precision loss
        inT_c = np.ascontiguousarray(inputs[bsl].reshape(BPC * L, HZ).T)
        f8 = inT_c.astype(NPF8)
        f83 = f8.reshape(HZ, BPC, L)
        img = np.zeros((128, ROWS // 2), dtype=NPF8)
        img[:HZ] = f83[:, 0::2].reshape(HZ, ROWS // 2)
        img[64 : 64 + HZ] = f83[:, 1::2].reshape(HZ, ROWS // 2)
        offs_c = (
            np.arange(H, dtype=np.int64)[None, :] * NUM_RATES
            + rate_indices[bsl].astype(np.int64)
        ).astype(np.int32)
        in_maps.append(
            {
                "in_img": img,
                "tau_tab": tau_tab,
                "offs": np.ascontiguousarray(offs_c),
                "bdvT": BDV_T,
                "bdw": BDW,
                "lam_rep": lam_rep,
            }
        )

    _NC_CACHE["in_maps"] = in_maps
    res = run_bass_kernel_spmd(nc, in_maps, core_ids=list(range(NCORES)))

    out = np.empty((B, L, H, K, S), dtype=np.float32)
    for c in range(NCORES):
        o = res.results[c]["out"]  # (80, 65536) bf16
        out[c * BPC : (c + 1) * BPC] = (
            o.astype(np.float32).T.reshape(BPC, L, H, K, S)
        )
    return out
